# revision 1
# baseline (speedup 1.0000x reference)
"""Trainium2 Bass kernel for LoRA multi-head causal attention (tensor-parallel
over heads across 8 NeuronCores).

Math (per reference):
  q = x@wq + (x@wq_A)@wq_B * 2 ; k,v analogous ; rope(q,k) ; causal softmax
  attention ; out = a@wo + (a@wo_A)@wo_B * 2

Device strategy (per core c, heads 2c and 2c+1):
  - LoRA folded into the dense weights on host (x@W + (x@A)@B*s == x@(W+s*A@B)).
  - q/k weights column-permuted per head (even rope pairs first) so rope is
    contiguous half-tile arithmetic; QK^T is invariant under a shared head-dim
    permutation.
  - Matmul operands in bf16 (PE 1 cyc/row + fast weight load); accumulation,
    softmax and normalization arithmetic in fp32 (PSUM).  The softmax
    normalizer path stays in float32r (full fp32-bit reciprocal).
  - Phase 1: Q^T,K^T (head-dim on partitions) + V (natural) from xT = x^T
    staged in DRAM; rope applied PSUM->SBUF (t2 computed in place in PSUM;
    mixed SBUF/PSUM operands may use different base partitions).
  - Phase 2: S^T = K^T.T @ Q^T per (batch,head,q-tile); exp on ScalarE
    (PSUM->SBUF bf16, scale=1/sqrt(hd)); multiplicative exp(mask) tiles on
    partially-masked blocks, fully-masked blocks skipped; P^T feeds PV and a
    ones-vector rowsum matmul; normalization = reciprocal + K=1 broadcast
    matmul.  The wo projection for each (batch, q-tile) is emitted right after
    its two heads finish so writeback overlaps attention.
  - Host sums the per-core partials (the tensor-parallel all-reduce).
"""
import sys
import math

sys.path.insert(0, "/opt/trn_rl_repo")

import numpy as np
import ml_dtypes

import concourse.bass as bass
from concourse import bacc
import concourse.mybir as mybir
from concourse.tile import TileContext
from concourse.bass_utils import run_bass_kernel_spmd

F32 = mybir.dt.float32
F32R = mybir.dt.float32r
BF16 = mybir.dt.bfloat16

B, S, D, H, R = 2, 2048, 2048, 16, 8
HD = D // H                     # 128
SCALING = 16.0 / R              # 2.0
N_CORES = 8
HPC = H // N_CORES              # heads per core = 2
DCOL = HPC * HD                 # per-core projection width = 256
SEQ = B * S                     # 4096
ISQ = 1.0 / math.sqrt(HD)
QTILE = 512                     # q-tile width (free dim)
KBLK = 128                      # k-block (partition dim)


def build_kernel(blocks, nm, kc, nsb, nqt, nkb, debug=False):
    """blocks: per q-tile (within a batch) list of (kt, mask_id|None)."""
    nc = bacc.Bacc("TRN2", target_bir_lowering=False, debug=False)

    xT = nc.declare_dram_parameter("xT", [D, SEQ], BF16, isOutput=False)
    wq = nc.declare_dram_parameter("wq", [D, DCOL], BF16, isOutput=False)
    wk = nc.declare_dram_parameter("wk", [D, DCOL], BF16, isOutput=False)
    wv = nc.declare_dram_parameter("wv", [D, DCOL], BF16, isOutput=False)
    wo = nc.declare_dram_parameter("wo", [DCOL, D], BF16, isOutput=False)
    cos2 = nc.declare_dram_parameter("cos2", [HD, S], F32, isOutput=False)
    sin2 = nc.declare_dram_parameter("sin2", [HD, S], F32, isOutput=False)
    masks = nc.declare_dram_parameter("masks", [nm * 128, QTILE], BF16, isOutput=False)
    ones = nc.declare_dram_parameter("ones", [128, 1], BF16, isOutput=False)
    onesrow = nc.declare_dram_parameter("onesrow", [1, 128], F32R, isOutput=False)
    outT = nc.declare_dram_parameter("outT", [D, SEQ], F32, isOutput=True)
    if debug:
        qt_d = nc.declare_dram_parameter("qt_d", [128, HPC * nsb * QTILE], BF16, isOutput=True)
        kt_d = nc.declare_dram_parameter("kt_d", [128, HPC * nsb * QTILE], BF16, isOutput=True)
        vs_d = nc.declare_dram_parameter("vs_d", [128, (SEQ // 128) * DCOL], BF16, isOutput=True)
        ot_d = nc.declare_dram_parameter("ot_d", [128, HPC * nsb * QTILE], BF16, isOutput=True)

    with TileContext(nc) as tc:
        from contextlib import ExitStack
        with ExitStack() as top:
            glob = top.enter_context(tc.tile_pool(name="glob", bufs=1))
            qkvs = top.enter_context(tc.tile_pool(name="qkvs", bufs=1))

            QT = qkvs.tile([128, HPC, nsb, QTILE], BF16, tag="QT")
            KT = qkvs.tile([128, HPC, nsb, QTILE], BF16, tag="KT")
            VS = qkvs.tile([128, SEQ // 128, DCOL], BF16, tag="VS")
            OT = qkvs.tile([128, HPC, nsb, QTILE], BF16, tag="OT")

            # ---------------- Phase 1: projections + rope ----------------
            with tc.tile_pool(name="wts", bufs=1) as wts, \
                 tc.tile_pool(name="xts", bufs=8) as xts, \
                 tc.tile_pool(name="tmp", bufs=3) as tmp, \
                 tc.tile_pool(name="ps1q", bufs=2, space="PSUM") as ps1q, \
                 tc.tile_pool(name="ps1", bufs=1, space="PSUM") as ps1:
                wq_t = wts.tile([128, kc, DCOL], BF16, tag="wq")
                wk_t = wts.tile([128, kc, DCOL], BF16, tag="wk")
                wv_t = wts.tile([128, kc, DCOL], BF16, tag="wv")

                sb_order = [x for pair in zip(range(nsb // 2), range(nsb // 2, nsb))
                            for x in pair] if nsb % 2 == 0 else list(range(nsb))
                for sb in sb_order:
                    # start=True clears the WHOLE psum bank -> one chain per
                    # psum tensor, EXCEPT the deliberate V bank-share below:
                    # the second region never uses start and rides the first
                    # region's bank clear (its has_written bits stay 0 until
                    # its own first write).
                    q_ps = [ps1q.tile([128, QTILE], F32, tag=f"q{h}", name=f"q_ps{h}") for h in range(HPC)]
                    k_ps = [ps1.tile([128, QTILE], F32, tag=f"k{h}", name=f"k_ps{h}") for h in range(HPC)]
                    v_ps = [ps1.tile([128, 2, DCOL], F32, tag=f"v{j}", name=f"v_ps{j}")
                            for j in range(2)]
                    for c in range(kc):
                        if sb == 0:
                            # stream weight chunks in first-use order so the
                            # first matmul isn't stuck behind the full preload
                            nc.sync.dma_start(out=wq_t[:, c, :], in_=wq[c * 128:(c + 1) * 128, :])
                            nc.sync.dma_start(out=wk_t[:, c, :], in_=wk[c * 128:(c + 1) * 128, :])
                            nc.sync.dma_start(out=wv_t[:, c, :], in_=wv[c * 128:(c + 1) * 128, :])
                        xt = xts.tile([128, QTILE], BF16, tag="xt")
                        nc.sync.dma_start(
                            out=xt, in_=xT[c * 128:(c + 1) * 128, sb * QTILE:(sb + 1) * QTILE])
                        st, sp = (c == 0), (c == kc - 1)
                        for h in range(HPC):
                            nc.tensor.matmul(q_ps[h], wq_t[:, c, h * HD:(h + 1) * HD], xt,
                                             start=st, stop=sp)
                        for sub in range(4):
                            nc.tensor.matmul(v_ps[sub // 2][:, sub % 2, :],
                                             xt[:, sub * 128:(sub + 1) * 128],
                                             wv_t[:, c, :],
                                             start=(st and sub % 2 == 0), stop=sp,
                                             skip_group_check=True)
                        for h in range(HPC):
                            nc.tensor.matmul(k_ps[h], wk_t[:, c, h * HD:(h + 1) * HD], xt,
                                             start=st, stop=sp)
                    if sb == 0:
                        cos2_t = glob.tile([HD, S], F32, tag="cos2")
                        nc.sync.dma_start(out=cos2_t, in_=cos2[:, :])
                        sin2_t = glob.tile([HD, S], F32, tag="sin2")
                        nc.sync.dma_start(out=sin2_t, in_=sin2[:, :])
                    # rope: psum [e;o] rows -> [re;im] rows in SBUF (bf16).
                    # ps*sin computed in place; half-cross combines rely on
                    # mixed SBUF/PSUM operands allowing different bases.
                    scol = (sb * QTILE) % S
                    cs = cos2_t[:, scol:scol + QTILE]
                    sn = sin2_t[:, scol:scol + QTILE]
                    # V copies first (free the shared banks the next sb's V
                    # matmuls need), then rope.  On the LAST sb the remaining
                    # attention chains are gated on this tile's Q/K, so rope
                    # head-0's K and Q first there.
                    for j in range(2):
                        nc.vector.tensor_copy(
                            VS[:, sb * 4 + 2 * j: sb * 4 + 2 * j + 2, :].rearrange("p a b -> p (a b)"),
                            v_ps[j].rearrange("p a b -> p (a b)"))
                    if sb == sb_order[-1]:
                        rope_order = [(0, k_ps[0], KT), (0, q_ps[0], QT),
                                      (1, k_ps[1], KT), (1, q_ps[1], QT)]
                    else:
                        rope_order = [(h, k_ps[h], KT) for h in range(HPC)] + \
                                     [(h, q_ps[h], QT) for h in range(HPC)]
                    for h, ps, dst in rope_order:
                        t1 = tmp.tile([128, QTILE], F32, tag="t1")
                        nc.vector.tensor_mul(t1, ps, cs)
                        nc.vector.tensor_mul(ps, ps, sn)
                        nc.vector.tensor_sub(dst[0:64, h, sb, :], t1[0:64, :], ps[64:128, :])
                        nc.vector.tensor_add(dst[64:128, h, sb, :], ps[0:64, :], t1[64:128, :])

            if debug:
                nc.sync.dma_start(out=qt_d[:, :], in_=QT.rearrange("p a b c -> p (a b c)"))
                nc.sync.dma_start(out=kt_d[:, :], in_=KT.rearrange("p a b c -> p (a b c)"))
                nc.sync.dma_start(out=vs_d[:, :], in_=VS.rearrange("p a b -> p (a b)"))

            # ---------------- Phase 2 + 3 interleaved ----------------
            with tc.tile_pool(name="wos", bufs=1) as wos, \
                 tc.tile_pool(name="pts", bufs=4) as pts, \
                 tc.tile_pool(name="rbs", bufs=2) as rbs, \
                 tc.tile_pool(name="outs", bufs=4) as outs, \
                 tc.tile_pool(name="ps_s", bufs=2, space="PSUM") as ps_s, \
                 tc.tile_pool(name="ps_o", bufs=1, space="PSUM") as ps_o, \
                 tc.tile_pool(name="ps_r", bufs=1, space="PSUM") as ps_r, \
                 tc.tile_pool(name="ps_b", bufs=1, space="PSUM") as ps_b, \
                 tc.tile_pool(name="ps3", bufs=1, space="PSUM") as ps3:
                wo_t = wos.tile([128, HPC, D], BF16, tag="wo")
                for j in range(HPC):
                    nc.sync.dma_start(out=wo_t[:, j, :], in_=wo[j * 128:(j + 1) * 128, :])
                ones_t = glob.tile([128, 1], BF16, tag="ones")
                nc.sync.dma_start(out=ones_t, in_=ones[:, :])
                onesr_t = glob.tile([1, 128], F32R, tag="onesr")
                nc.sync.dma_start(out=onesr_t, in_=onesrow[:, :])
                masks_t = glob.tile([128, nm, QTILE], BF16, tag="masks")
                for m in range(nm):
                    nc.sync.dma_start(out=masks_t[:, m, :], in_=masks[m * 128:(m + 1) * 128, :])

                # Deferred-op scheduler: PE-queue-blocking pieces of the
                # normalization / wo projection are emitted a few chain-steps
                # after their inputs start, so the in-order PE stream never
                # waits on the DVE reciprocal or the OT normalization.
                import heapq
                todo = []      # (due_step, seq, fn)
                gstep = 0
                seq = [0]

                def sched(delay, fn):
                    heapq.heappush(todo, (gstep + delay, seq[0], fn))
                    seq[0] += 1

                def emit_due():
                    while todo and todo[0][0] <= gstep:
                        heapq.heappop(todo)[2]()

                def norm_a(h, sbq, po, pr):
                    # chain end: free po/pr fast, compute reciprocal (DVE only)
                    nc.vector.tensor_copy(OT[:, h, sbq, :], po)
                    rpf = rbs.tile([1, QTILE], F32, tag="rpf")
                    nc.vector.reciprocal_approx_fast(rpf, pr)
                    rp = rbs.tile([1, QTILE], F32R, tag="rp")
                    with nc.allow_low_precision(reason="fp32r bits are fp32"):
                        nc.vector.tensor_copy(rp, rpf)
                    return rp

                def norm_b(rp):
                    rb_ps = ps_b.tile([128, QTILE], F32, tag="rb")
                    nc.tensor.matmul(rb_ps, onesr_t[:, :], rp[:, :],
                                     start=True, stop=True)
                    rb = rbs.tile([128, QTILE], F32, tag="rbs")
                    nc.scalar.copy(rb, rb_ps)
                    return rb

                def norm_c(h, sbq, rb):
                    nc.vector.tensor_mul(OT[:, h, sbq, :], OT[:, h, sbq, :], rb)

                def wo_part(sbq, dc0, ndc):
                    for dc in range(dc0, dc0 + ndc):
                        po3 = ps3.tile([128, QTILE], F32, tag="po3")
                        for j in range(HPC):
                            nc.tensor.matmul(po3, wo_t[:, j, dc * 128:(dc + 1) * 128],
                                             OT[:, j, sbq, :],
                                             start=(j == 0), stop=(j == HPC - 1))
                        os3 = outs.tile([128, QTILE], F32, tag="os3")
                        nc.any.tensor_copy(os3, po3)
                        nc.sync.dma_start(
                            out=outT[dc * 128:(dc + 1) * 128,
                                     sbq * QTILE:(sbq + 1) * QTILE],
                            in_=os3)

                def norm_chain(h, sbq, po, pr, is_last_head):
                    rp = norm_a(h, sbq, po, pr)

                    def _b():
                        rb = norm_b(rp)

                        def _c():
                            norm_c(h, sbq, rb)
                            if is_last_head:
                                for w in range(4):
                                    sched(1 + w, lambda w=w: wo_part(sbq, w * 4, 4))
                        sched(1, _c)
                    sched(2, _b)

                bqt_order = [(b, qt) for qt in range(nqt) for b in range(B)]
                for b, qt in bqt_order:
                    {}.clear()  # no-op to keep indentation structure
                    if True:
                        sbq = (b * S) // QTILE + qt
                        blist = blocks[qt]
                        nbl = len(blist)
                        for h in range(HPC):
                            po = ps_o.tile([128, QTILE], F32, tag="po")
                            pr = ps_r.tile([1, QTILE], F32, tag="pr")
                            # process score blocks in pairs: two QK matmuls into
                            # a 2-bank psum tile, ONE [128,1024] exp instruction
                            for p0 in range(0, nbl, 2):
                                pair = blist[p0:p0 + 2]
                                s_big = ps_s.tile([128, 2, QTILE], F32, tag="s")
                                for j, (kt, mid) in enumerate(pair):
                                    sbk = (b * S + kt * 128) // QTILE
                                    ck = (kt * 128) % QTILE
                                    nc.tensor.matmul(s_big[:, j, :],
                                                     KT[:, h, sbk, ck:ck + 128],
                                                     QT[:, h, sbq, :],
                                                     start=True, stop=True)
                                pt2 = pts.tile([128, 2, QTILE], BF16, tag="pt")
                                if len(pair) == 2:
                                    nc.scalar.activation(
                                        pt2.rearrange("p a b -> p (a b)"),
                                        s_big.rearrange("p a b -> p (a b)"),
                                        mybir.ActivationFunctionType.Exp,
                                        scale=float(ISQ))
                                else:
                                    nc.scalar.activation(
                                        pt2[:, 0, :], s_big[:, 0, :],
                                        mybir.ActivationFunctionType.Exp,
                                        scale=float(ISQ))
                                for j, (kt, mid) in enumerate(pair):
                                    i = p0 + j
                                    if mid is not None:
                                        nc.vector.tensor_mul(pt2[:, j, :], pt2[:, j, :],
                                                             masks_t[:, mid, :])
                                    gkt = (b * S) // 128 + kt
                                    nc.tensor.matmul(po, VS[:, gkt, h * HD:(h + 1) * HD],
                                                     pt2[:, j, :],
                                                     start=(i == 0), stop=(i == nbl - 1))
                                    nc.tensor.matmul(pr, ones_t[:, :], pt2[:, j, :],
                                                     start=(i == 0), stop=(i == nbl - 1))
                                    gstep += 1
                                    emit_due()
                            norm_chain(h, sbq, po, pr, h == HPC - 1)
                while todo:
                    gstep += 1
                    emit_due()
                if debug:
                    nc.sync.dma_start(out=ot_d[:, :], in_=OT.rearrange("p a b c -> p (a b c)"))
    nc.compile()
    return nc


# ---------------------------------------------------------------------------
# Host-side preparation
# ---------------------------------------------------------------------------

_CACHE = {}


def _classify_blocks(mask):
    """mask: additive [S, S] (q, k) -> (blocks, mask_tiles[128*nm, QTILE])."""
    nqt, nkb = S // QTILE, S // KBLK
    mult = np.exp(np.minimum(mask, 0.0).astype(np.float64)).astype(np.float32)
    blocks = []
    tiles = []
    tile_index = {}
    for qt in range(nqt):
        row = []
        qs = slice(qt * QTILE, (qt + 1) * QTILE)
        for kt in range(nkb):
            ks = slice(kt * KBLK, (kt + 1) * KBLK)
            blk = mult[qs, ks].T  # [k, q]
            if not blk.any():
                continue
            if (blk == 1.0).all():
                row.append((kt, None))
                continue
            key = blk.tobytes()
            if key not in tile_index:
                tile_index[key] = len(tiles)
                tiles.append(np.ascontiguousarray(blk))
            row.append((kt, tile_index[key]))
        assert row, "fully-masked q-tile row: softmax undefined in this kernel"
        blocks.append(row)
    if not tiles:
        tiles.append(np.zeros((KBLK, QTILE), np.float32))
    return blocks, np.concatenate(tiles, axis=0)


def _perm_even_odd():
    p = np.empty(HD, np.int64)
    p[:64] = np.arange(0, HD, 2)
    p[64:] = np.arange(1, HD, 2)
    return p


def _bf16(a):
    return np.ascontiguousarray(np.asarray(a, np.float32).astype(ml_dtypes.bfloat16))


def kernel(x, wq, wk, wv, wo, wq_A, wq_B, wk_A, wk_B, wv_A, wv_B,
           wo_A, wo_B, cos, sin, mask):
    x = np.asarray(x, np.float32)
    to64 = lambda a: np.asarray(a, np.float32).astype(np.float64)

    wq_f = (to64(wq) + SCALING * (to64(wq_A) @ to64(wq_B))).astype(np.float32)
    wk_f = (to64(wk) + SCALING * (to64(wk_A) @ to64(wk_B))).astype(np.float32)
    wv_f = (to64(wv) + SCALING * (to64(wv_A) @ to64(wv_B))).astype(np.float32)
    wo_f = (to64(wo) + SCALING * (to64(wo_A) @ to64(wo_B))).astype(np.float32)

    perm = _perm_even_odd()
    full_perm = np.concatenate([h * HD + perm for h in range(H)])
    wq_p = wq_f[:, full_perm]
    wk_p = wk_f[:, full_perm]

    cosT = np.ascontiguousarray(np.asarray(cos, np.float32).T)  # [64, S]
    sinT = np.ascontiguousarray(np.asarray(sin, np.float32).T)
    cos2 = np.concatenate([cosT, cosT], axis=0)  # [128, S]
    sin2 = np.concatenate([sinT, sinT], axis=0)

    m2d = np.asarray(mask, np.float32).reshape(S, S)
    blocks, mask_tiles = _classify_blocks(m2d)
    nm = mask_tiles.shape[0] // 128

    sig = (tuple(tuple(r) for r in blocks), nm)
    if sig not in _CACHE:
        _CACHE[sig] = build_kernel(blocks, nm, D // 128, SEQ // QTILE,
                                   S // QTILE, S // KBLK)
    nc = _CACHE[sig]

    xT = _bf16(x.reshape(SEQ, D).T)
    ones = np.ones((128, 1), ml_dtypes.bfloat16)
    onesrow = np.ones((1, 128), np.float32)

    in_maps = []
    for c in range(N_CORES):
        cols = slice(c * DCOL, (c + 1) * DCOL)
        in_maps.append(dict(
            xT=xT,
            wq=_bf16(wq_p[:, cols]),
            wk=_bf16(wk_p[:, cols]),
            wv=_bf16(wv_f[:, cols]),
            wo=_bf16(wo_f[cols, :]),
            cos2=cos2, sin2=sin2, masks=_bf16(mask_tiles),
            ones=ones, onesrow=onesrow,
        ))

    global _LAST
    res = run_bass_kernel_spmd(nc, in_maps, list(range(N_CORES)), trace=_TRACE)
    _LAST = res
    acc = np.zeros((D, SEQ), np.float32)
    for r in res.results:
        acc += r["outT"]
    return np.ascontiguousarray(acc.T).reshape(B, S, D)


_TRACE = False   # test harness can set kernel._TRACE = True to profile
_LAST = None     # last BassKernelResults (exec_time_ns when traced)



# revision 7
# speedup vs baseline: 1.1242x; 1.1242x over previous
"""Trainium2 Bass kernel for LoRA multi-head causal attention (tensor-parallel
over heads across 8 NeuronCores).

Math (per reference):
  q = x@wq + (x@wq_A)@wq_B * 2 ; k,v analogous ; rope(q,k) ; causal softmax
  attention ; out = a@wo + (a@wo_A)@wo_B * 2

Device strategy (per core c, heads 2c and 2c+1):
  - LoRA folded into the dense weights on host (x@W + (x@A)@B*s == x@(W+s*A@B)).
  - q/k weights column-permuted per head (even rope pairs first) so rope is
    contiguous half-tile arithmetic; QK^T is invariant under a shared head-dim
    permutation.
  - Matmul operands bf16; accumulation/softmax fp32 in PSUM; normalizer path
    float32r.
  - Phase 1 (per 512-token sb tile, b-major): three separated chunk loops
    Q(c0..15), K(c0..15), V(c0..15) so each PSUM consumer (rope on DVE, V
    copies) drains while the PE streams the next loop.  QT/KT/VS are per-sb
    tiles so phase-2 dependencies are precise; PSUM pool creation order makes
    phase-2 banks alias the earliest-freed phase-1 banks.
  - DMA descriptor count minimized (sync-engine descriptor programming is
    ~600ns each and was the phase-1 supply bottleneck): weights one
    multi-chunk strided descriptor per kind, xt in 4-chunk x 1024-col groups,
    outputs in 4-block groups; issue order hand-arranged for the cold start.
  - Phase 2: flat stream over all (b,qt,h,block) units with lookahead-2
    software pipelining: scores for block i+2 are emitted before PV/rowsum of
    block i, so the PE never waits on the ScalarE exp.  Single-block exp
    (PSUM->SBUF bf16, scale=1/sqrt(hd)); multiplicative mask tiles on partial
    blocks; rowsum via ones-vector matmul; normalization reciprocal on DVE,
    broadcast via K=1 matmul; wo projection interleaved via deferred
    scheduler; partial outputs written bf16 (host does the fp32 all-reduce).
"""
import sys
import math

sys.path.insert(0, "/opt/trn_rl_repo")

import numpy as np
import ml_dtypes

import concourse.bass as bass
from concourse import bacc
import concourse.mybir as mybir
from concourse.tile import TileContext
from concourse.bass_utils import run_bass_kernel_spmd

F32 = mybir.dt.float32
F32R = mybir.dt.float32r
BF16 = mybir.dt.bfloat16

B, S, D, H, R = 2, 2048, 2048, 16, 8
HD = D // H                     # 128
SCALING = 16.0 / R              # 2.0
N_CORES = 8
HPC = H // N_CORES              # heads per core = 2
DCOL = HPC * HD                 # per-core projection width = 256
SEQ = B * S                     # 4096
ISQ = 1.0 / math.sqrt(HD)
QTILE = 512                     # q-tile width (free dim)
KBLK = 128                      # k-block (partition dim)
KC = D // 128                   # 16 contraction chunks
NSB = SEQ // QTILE              # 8 sb tiles
NQT = S // QTILE                # 4 q tiles per batch
LA = 2                          # phase-2 score lookahead (blocks)
XG = 4                          # xt chunks per DMA group


def build_kernel(blocks, nm):
    nc = bacc.Bacc("TRN2", target_bir_lowering=False, debug=False)

    xT = nc.declare_dram_parameter("xT", [D, SEQ], BF16, isOutput=False)
    wqkv = nc.declare_dram_parameter("wqkv", [3, KC, 128, DCOL], BF16, isOutput=False)
    wo = nc.declare_dram_parameter("wo", [HPC, 128, D], BF16, isOutput=False)
    cos2 = nc.declare_dram_parameter("cos2", [HD, S], F32, isOutput=False)
    sin2 = nc.declare_dram_parameter("sin2", [HD, S], F32, isOutput=False)
    masks = nc.declare_dram_parameter("masks", [nm, 128, QTILE], BF16, isOutput=False)
    ones = nc.declare_dram_parameter("ones", [128, 1], BF16, isOutput=False)
    onesrow = nc.declare_dram_parameter("onesrow", [1, 128], F32R, isOutput=False)
    outT = nc.declare_dram_parameter("outT", [KC, 128, SEQ], BF16, isOutput=True)

    with TileContext(nc) as tc:
        from contextlib import ExitStack
        with ExitStack() as top:
            glob = top.enter_context(tc.tile_pool(name="glob", bufs=1))
            qkvs = top.enter_context(tc.tile_pool(name="qkvs", bufs=1))

            QTs = [qkvs.tile([128, HPC, QTILE], BF16, tag=f"QT{sb}", name=f"QT{sb}")
                   for sb in range(NSB)]
            KTs = [qkvs.tile([128, HPC, QTILE], BF16, tag=f"KT{sb}", name=f"KT{sb}")
                   for sb in range(NSB)]
            VSs = [qkvs.tile([128, QTILE // 128, DCOL], BF16, tag=f"VS{sb}", name=f"VS{sb}")
                   for sb in range(NSB)]
            OTs = [qkvs.tile([128, HPC, QTILE], BF16, tag=f"OT{sb}", name=f"OT{sb}")
                   for sb in range(NSB)]

            cos2_t = glob.tile([HD, S], F32, tag="cos2")
            sin2_t = glob.tile([HD, S], F32, tag="sin2")
            wo_t = glob.tile([128, HPC, D], BF16, tag="wo")
            masks_t = glob.tile([128, nm, QTILE], BF16, tag="masks")
            ones_t = glob.tile([128, 1], BF16, tag="ones")
            onesr_t = glob.tile([1, 128], F32R, tag="onesr")

            xT3 = xT.rearrange("(c p) s -> p c s", p=128)

            # ---------------- Phase 1: projections + rope ----------------
            # ps1 (k/v) created before ps1q (q) so phase-2 pools alias the
            # banks freed earliest at the tail (k ropes run during sb7's V
            # loop; q ropes are the last DVE ops).
            with tc.tile_pool(name="wts", bufs=1) as wts, \
                 tc.tile_pool(name="xts", bufs=8) as xts, \
                 tc.tile_pool(name="tmp", bufs=3) as tmp, \
                 tc.tile_pool(name="ps1", bufs=1, space="PSUM") as ps1, \
                 tc.tile_pool(name="ps1q", bufs=2, space="PSUM") as ps1q:
                wqkv_t = wts.tile([128, 3, KC, DCOL], BF16, tag="wqkv")

                def load_w(kind):
                    nc.sync.dma_start(
                        out=wqkv_t[:, kind],
                        in_=wqkv[kind].rearrange("c p w -> p c w"))

                xt_tiles = {}   # pair -> list of group tiles [128, XG, 1024]

                def load_group(pair, g):
                    t = xts.tile([128, XG, 2 * QTILE], BF16, tag="xt", name="xt")
                    nc.sync.dma_start(
                        out=t,
                        in_=xT3[:, g * XG:(g + 1) * XG,
                                pair * 2 * QTILE:(pair + 1) * 2 * QTILE])
                    xt_tiles.setdefault(pair, []).append(t)

                # cold-start critical DMA order
                load_w(0)
                load_group(0, 0)
                load_w(1)
                load_group(0, 1)
                nc.sync.dma_start(out=cos2_t, in_=cos2[:, :])
                nc.sync.dma_start(out=sin2_t, in_=sin2[:, :])
                load_group(0, 2)
                load_group(0, 3)
                load_w(2)
                nc.sync.dma_start(out=wo_t, in_=wo.rearrange("j p d -> p j d"))
                nc.sync.dma_start(out=masks_t,
                                  in_=masks.rearrange("m p q -> p m q"))
                nc.sync.dma_start(out=ones_t, in_=ones[:, :])
                nc.sync.dma_start(out=onesr_t, in_=onesrow[:, :])

                for sb in range(NSB):
                    pair, half = sb // 2, sb % 2
                    if half == 0 and pair + 1 < NSB // 2:
                        for g in range(KC // XG):
                            load_group(pair + 1, g)
                    xg = xt_tiles[pair]

                    def xsl(c):
                        return xg[c // XG][:, c % XG, half * QTILE:(half + 1) * QTILE]

                    k_ps = [ps1.tile([128, QTILE], F32, tag=f"k{h}", name=f"k_ps{h}")
                            for h in range(HPC)]
                    v_ps = [ps1.tile([128, 2, DCOL], F32, tag=f"v{j}", name=f"v_ps{j}")
                            for j in range(2)]
                    q_ps = [ps1q.tile([128, QTILE], F32, tag=f"q{h}", name=f"q_ps{h}")
                            for h in range(HPC)]

                    scol = (sb * QTILE) % S
                    cs = cos2_t[:, scol:scol + QTILE]
                    sn = sin2_t[:, scol:scol + QTILE]

                    def rope(ps, dst):
                        t1 = tmp.tile([128, QTILE], F32, tag="t1", name="t1")
                        nc.vector.tensor_mul(t1, ps, cs)
                        nc.vector.tensor_mul(ps, ps, sn)
                        nc.vector.tensor_sub(dst[0:64, :], t1[0:64, :], ps[64:128, :])
                        nc.vector.tensor_add(dst[64:128, :], ps[0:64, :], t1[64:128, :])

                    # Q loop
                    for c in range(KC):
                        for h in range(HPC):
                            nc.tensor.matmul(q_ps[h],
                                             wqkv_t[:, 0, c, h * HD:(h + 1) * HD],
                                             xsl(c), start=(c == 0), stop=(c == KC - 1))
                    # K loop
                    for c in range(KC):
                        for h in range(HPC):
                            nc.tensor.matmul(k_ps[h],
                                             wqkv_t[:, 1, c, h * HD:(h + 1) * HD],
                                             xsl(c), start=(c == 0), stop=(c == KC - 1))
                    # K rope drains k_ps on DVE while the PE streams the V loop
                    for h in range(HPC):
                        rope(k_ps[h], KTs[sb][:, h, :])
                    # V loop (xt slices as stationary -> natural-layout V)
                    for c in range(KC):
                        x512 = xsl(c)
                        for sub in range(4):
                            nc.tensor.matmul(v_ps[sub // 2][:, sub % 2, :],
                                             x512[:, sub * 128:(sub + 1) * 128],
                                             wqkv_t[:, 2, c, :],
                                             start=(c == 0 and sub % 2 == 0),
                                             stop=(c == KC - 1),
                                             skip_group_check=True)
                    for j in range(2):
                        nc.vector.tensor_copy(
                            VSs[sb][:, 2 * j:2 * j + 2, :].rearrange("p a b -> p (a b)"),
                            v_ps[j].rearrange("p a b -> p (a b)"))
                    for h in range(HPC):
                        rope(q_ps[h], QTs[sb][:, h, :])

            # ---------------- Phase 2 + 3 interleaved ----------------
            with tc.tile_pool(name="pts", bufs=4) as pts, \
                 tc.tile_pool(name="rbs", bufs=2) as rbs, \
                 tc.tile_pool(name="outs", bufs=3) as outs, \
                 tc.tile_pool(name="ps_s", bufs=3, space="PSUM") as ps_s, \
                 tc.tile_pool(name="ps_o", bufs=2, space="PSUM") as ps_o, \
                 tc.tile_pool(name="ps_r", bufs=1, space="PSUM") as ps_r, \
                 tc.tile_pool(name="ps3", bufs=2, space="PSUM") as ps3:

                # Deferred-op scheduler (PE-queue-blocking pieces emitted a few
                # units after their inputs start so the in-order PE stream
                # never waits on DVE/ACT latency).
                import heapq
                todo = []
                gstep = 0
                seq = [0]

                def sched(delay, fn):
                    heapq.heappush(todo, (gstep + delay, seq[0], fn))
                    seq[0] += 1

                def emit_due():
                    while todo and todo[0][0] <= gstep:
                        heapq.heappop(todo)[2]()

                def wo_part(sbq, w):
                    os = outs.tile([128, 4, QTILE], BF16, tag="os", name="os")
                    for i in range(4):
                        dc = w * 4 + i
                        po3 = ps3.tile([128, QTILE], F32, tag="x", name="po3")
                        for j in range(HPC):
                            nc.tensor.matmul(po3, wo_t[:, j, dc * 128:(dc + 1) * 128],
                                             OTs[sbq][:, j, :],
                                             start=(j == 0), stop=(j == HPC - 1))
                        nc.any.tensor_copy(os[:, i, :], po3)
                    nc.sync.dma_start(
                        out=outT[w * 4:(w + 1) * 4, :, sbq * QTILE:(sbq + 1) * QTILE]
                            .rearrange("g p s -> p g s"),
                        in_=os)

                def norm_chain(h, sbq, po, pr, is_last_head):
                    # reciprocal first: frees pr for the next chain's rowsum
                    rpf = rbs.tile([1, QTILE], F32, tag="rpf", name="rpf")
                    nc.vector.reciprocal_approx_fast(rpf, pr)
                    rp = rbs.tile([1, QTILE], F32R, tag="rp", name="rp")
                    with nc.allow_low_precision(reason="fp32r bits are fp32"):
                        nc.vector.tensor_copy(rp, rpf)

                    def _b():
                        rb_ps = ps3.tile([128, QTILE], F32, tag="x", name="rb_ps")
                        nc.tensor.matmul(rb_ps, onesr_t[:, :], rp[:, :],
                                         start=True, stop=True)
                        rb = rbs.tile([128, QTILE], F32, tag="rb", name="rb")
                        nc.scalar.copy(rb, rb_ps)

                        def _c():
                            nc.vector.tensor_mul(OTs[sbq][:, h, :], po, rb)
                            if is_last_head:
                                for w in range(4):
                                    sched(1 + w, lambda w=w: wo_part(sbq, w))
                        sched(1, _c)
                    sched(1, _b)

                # flat unit stream over all (b,qt,h,block); ends on a short
                # chain so the un-overlapped tail is small
                bqt_order = [(0, 0), (0, 1), (0, 2), (0, 3),
                             (1, 1), (1, 2), (1, 3), (1, 0)]
                units = []
                for b, qt in bqt_order:
                    sbq = b * NQT + qt
                    blist = blocks[qt]
                    for h in range(HPC):
                        for i, (kt, mid) in enumerate(blist):
                            units.append((b, qt, sbq, h, i, kt, mid, len(blist)))

                state = {}      # (sbq,h) -> (po, pr)
                pt_of = {}      # unit idx -> pt tile

                def emit_score(idx):
                    b, qt, sbq, h, i, kt, mid, nbl = units[idx]
                    sbk = (b * S + kt * 128) // QTILE
                    ck = (kt * 128) % QTILE
                    s_ps = ps_s.tile([128, QTILE], F32, tag="s", name="s_ps")
                    nc.tensor.matmul(s_ps, KTs[sbk][:, h, ck:ck + 128],
                                     QTs[sbq][:, h, :], start=True, stop=True)
                    pt = pts.tile([128, QTILE], BF16, tag="pt", name="pt")
                    nc.scalar.activation(pt, s_ps,
                                         mybir.ActivationFunctionType.Exp,
                                         scale=float(ISQ))
                    if mid is not None:
                        nc.vector.tensor_mul(pt, pt, masks_t[:, mid, :])
                    pt_of[idx] = pt

                def emit_pv(idx):
                    b, qt, sbq, h, i, kt, mid, nbl = units[idx]
                    if i == 0:
                        po = ps_o.tile([128, QTILE], F32, tag="po", name="po")
                        pr = ps_r.tile([1, QTILE], F32, tag="pr", name="pr")
                        state[(sbq, h)] = (po, pr)
                    po, pr = state[(sbq, h)]
                    pt = pt_of.pop(idx)
                    gkt = (b * S) // 128 + kt
                    nc.tensor.matmul(po, VSs[gkt // 4][:, gkt % 4, h * HD:(h + 1) * HD],
                                     pt, start=(i == 0), stop=(i == nbl - 1))
                    nc.tensor.matmul(pr, ones_t[:, :], pt,
                                     start=(i == 0), stop=(i == nbl - 1))
                    if i == nbl - 1:
                        norm_chain(h, sbq, po, pr, h == HPC - 1)

                for idx in range(len(units) + LA):
                    if idx < len(units):
                        emit_score(idx)
                    j = idx - LA
                    if j >= 0:
                        emit_pv(j)
                        gstep += 1
                        emit_due()
                while todo:
                    gstep += 1
                    emit_due()
    nc.compile()
    return nc


# ---------------------------------------------------------------------------
# Host-side preparation
# ---------------------------------------------------------------------------

_CACHE = {}


def _classify_blocks(mask):
    """mask: additive [S, S] (q, k) -> (blocks, mask_tiles[nm, 128, QTILE])."""
    nqt, nkb = S // QTILE, S // KBLK
    mult = np.exp(np.minimum(mask, 0.0).astype(np.float64)).astype(np.float32)
    blocks = []
    tiles = []
    tile_index = {}
    for qt in range(nqt):
        row = []
        qs = slice(qt * QTILE, (qt + 1) * QTILE)
        for kt in range(nkb):
            ks = slice(kt * KBLK, (kt + 1) * KBLK)
            blk = mult[qs, ks].T  # [k, q]
            if not blk.any():
                continue
            if (blk == 1.0).all():
                row.append((kt, None))
                continue
            key = blk.tobytes()
            if key not in tile_index:
                tile_index[key] = len(tiles)
                tiles.append(np.ascontiguousarray(blk))
            row.append((kt, tile_index[key]))
        assert row, "fully-masked q-tile row: softmax undefined in this kernel"
        blocks.append(row)
    if not tiles:
        tiles.append(np.zeros((KBLK, QTILE), np.float32))
    return blocks, np.stack(tiles, axis=0)


def _perm_even_odd():
    p = np.empty(HD, np.int64)
    p[:64] = np.arange(0, HD, 2)
    p[64:] = np.arange(1, HD, 2)
    return p


def _bf16(a):
    return np.ascontiguousarray(np.asarray(a, np.float32).astype(ml_dtypes.bfloat16))


def kernel(x, wq, wk, wv, wo, wq_A, wq_B, wk_A, wk_B, wv_A, wv_B,
           wo_A, wo_B, cos, sin, mask):
    x = np.asarray(x, np.float32)
    to64 = lambda a: np.asarray(a, np.float32).astype(np.float64)

    wq_f = (to64(wq) + SCALING * (to64(wq_A) @ to64(wq_B))).astype(np.float32)
    wk_f = (to64(wk) + SCALING * (to64(wk_A) @ to64(wk_B))).astype(np.float32)
    wv_f = (to64(wv) + SCALING * (to64(wv_A) @ to64(wv_B))).astype(np.float32)
    wo_f = (to64(wo) + SCALING * (to64(wo_A) @ to64(wo_B))).astype(np.float32)

    perm = _perm_even_odd()
    full_perm = np.concatenate([h * HD + perm for h in range(H)])
    wq_p = wq_f[:, full_perm]
    wk_p = wk_f[:, full_perm]

    cosT = np.ascontiguousarray(np.asarray(cos, np.float32).T)  # [64, S]
    sinT = np.ascontiguousarray(np.asarray(sin, np.float32).T)
    cos2 = np.concatenate([cosT, cosT], axis=0)  # [128, S]
    sin2 = np.concatenate([sinT, sinT], axis=0)

    m2d = np.asarray(mask, np.float32).reshape(S, S)
    blocks, mask_tiles = _classify_blocks(m2d)
    nm = mask_tiles.shape[0]

    sig = (tuple(tuple(r) for r in blocks), nm)
    if sig not in _CACHE:
        _CACHE[sig] = build_kernel(blocks, nm)
    nc = _CACHE[sig]

    xT = _bf16(x.reshape(SEQ, D).T)
    ones = np.ones((128, 1), ml_dtypes.bfloat16)
    onesrow = np.ones((1, 128), np.float32)

    in_maps = []
    for c in range(N_CORES):
        cols = slice(c * DCOL, (c + 1) * DCOL)
        wqkv = np.stack([
            _bf16(wq_p[:, cols]).reshape(KC, 128, DCOL),
            _bf16(wk_p[:, cols]).reshape(KC, 128, DCOL),
            _bf16(wv_f[:, cols]).reshape(KC, 128, DCOL),
        ], axis=0)  # [3, KC, 128, DCOL]
        in_maps.append(dict(
            xT=xT,
            wqkv=wqkv,
            wo=_bf16(wo_f[cols, :]).reshape(HPC, 128, D),
            cos2=cos2, sin2=sin2, masks=_bf16(mask_tiles),
            ones=ones, onesrow=onesrow,
        ))

    global _LAST
    res = run_bass_kernel_spmd(nc, in_maps, list(range(N_CORES)), trace=_TRACE)
    _LAST = res
    acc = np.zeros((D, SEQ), np.float64)
    for r in res.results:
        acc += r["outT"].astype(np.float64).reshape(D, SEQ)
    return np.ascontiguousarray(acc.T.astype(np.float32)).reshape(B, S, D)


_TRACE = False   # test harness can set kernel._TRACE = True to profile
_LAST = None     # last BassKernelResults (exec_time_ns when traced)


# revision 11
# speedup vs baseline: 1.1755x; 1.0457x over previous
"""Trainium2 Bass kernel for LoRA multi-head causal attention (tensor-parallel
over heads across 8 NeuronCores).

Math (per reference):
  q = x@wq + (x@wq_A)@wq_B * 2 ; k,v analogous ; rope(q,k) ; causal softmax
  attention ; out = a@wo + (a@wo_A)@wo_B * 2

Device strategy (per core c, heads 2c and 2c+1):
  - LoRA folded into the dense weights on host (x@W + (x@A)@B*s == x@(W+s*A@B)).
  - q/k weights column-permuted per head (even rope pairs first) so rope is
    contiguous half-tile arithmetic; QK^T is invariant under a shared head-dim
    permutation.
  - Matmul operands bf16; accumulation/softmax fp32 in PSUM; normalizer path
    float32r.
  - Phase 1 (per 512-token sb tile, b-major): three separated chunk loops
    Q(c0..15), K(c0..15), V(c0..15) so each PSUM consumer (rope on DVE, V
    copies) drains while the PE streams the next loop.  QT/KT/VS are per-sb
    tiles so phase-2 dependencies are precise; PSUM pool creation order makes
    phase-2 banks alias the earliest-freed phase-1 banks.
  - DMA descriptor count minimized (sync-engine descriptor programming is
    ~600ns each and was the phase-1 supply bottleneck): weights one
    multi-chunk strided descriptor per kind, xt in 4-chunk x 1024-col groups,
    outputs in 4-block groups; issue order hand-arranged for the cold start.
  - Phase 2: flat stream over all (b,qt,h,block) units with lookahead-2
    software pipelining: scores for block i+2 are emitted before PV/rowsum of
    block i, so the PE never waits on the ScalarE exp.  Single-block exp
    (PSUM->SBUF bf16, scale=1/sqrt(hd)); multiplicative mask tiles on partial
    blocks; rowsum via ones-vector matmul; normalization reciprocal on DVE,
    broadcast via K=1 matmul; wo projection interleaved via deferred
    scheduler; partial outputs written bf16 (host does the fp32 all-reduce).
"""
import sys
import math

sys.path.insert(0, "/opt/trn_rl_repo")

import numpy as np
import ml_dtypes

import concourse.bass as bass
from concourse import bacc
import concourse.mybir as mybir
from concourse.tile import TileContext
from concourse.bass_utils import run_bass_kernel_spmd

F32 = mybir.dt.float32
F32R = mybir.dt.float32r
BF16 = mybir.dt.bfloat16

B, S, D, H, R = 2, 2048, 2048, 16, 8
HD = D // H                     # 128
SCALING = 16.0 / R              # 2.0
N_CORES = 8
HPC = H // N_CORES              # heads per core = 2
DCOL = HPC * HD                 # per-core projection width = 256
SEQ = B * S                     # 4096
ISQ = 1.0 / math.sqrt(HD)
QTILE = 512                     # q-tile width (free dim)
KBLK = 128                      # k-block (partition dim)
KC = D // 128                   # 16 contraction chunks
NSB = SEQ // QTILE              # 8 sb tiles
NQT = S // QTILE                # 4 q tiles per batch
LA = 2                          # phase-2 score lookahead (blocks)
XG = 4                          # xt chunks per DMA group


def build_kernel(blocks, nm):
    nc = bacc.Bacc("TRN2", target_bir_lowering=False, debug=False)

    xT = nc.declare_dram_parameter("xT", [D, SEQ], BF16, isOutput=False)
    wqkv = nc.declare_dram_parameter("wqkv", [3, KC, 128, DCOL], BF16, isOutput=False)
    wo = nc.declare_dram_parameter("wo", [HPC, 128, D], BF16, isOutput=False)
    cos2 = nc.declare_dram_parameter("cos2", [HD, S], F32, isOutput=False)
    sin2 = nc.declare_dram_parameter("sin2", [HD, S], F32, isOutput=False)
    masks = nc.declare_dram_parameter("masks", [nm, 128, QTILE], BF16, isOutput=False)
    ones = nc.declare_dram_parameter("ones", [128, 1], BF16, isOutput=False)
    onesrow = nc.declare_dram_parameter("onesrow", [1, 128], F32R, isOutput=False)
    outT = nc.declare_dram_parameter("outT", [KC, 128, SEQ], BF16, isOutput=True)

    with TileContext(nc) as tc:
        from contextlib import ExitStack
        with ExitStack() as top:
            glob = top.enter_context(tc.tile_pool(name="glob", bufs=1))
            qkvs = top.enter_context(tc.tile_pool(name="qkvs", bufs=1))

            QTs = [qkvs.tile([128, HPC, QTILE], BF16, tag=f"QT{sb}", name=f"QT{sb}")
                   for sb in range(NSB)]
            KTs = [qkvs.tile([128, HPC, QTILE], BF16, tag=f"KT{sb}", name=f"KT{sb}")
                   for sb in range(NSB)]
            VSs = [qkvs.tile([128, QTILE // 128, DCOL], BF16, tag=f"VS{sb}", name=f"VS{sb}")
                   for sb in range(NSB)]
            OTs = [qkvs.tile([128, HPC, QTILE], BF16, tag=f"OT{sb}", name=f"OT{sb}")
                   for sb in range(NSB)]

            cos2_t = glob.tile([HD, S], F32, tag="cos2")
            sin2_t = glob.tile([HD, S], F32, tag="sin2")
            wo_t = glob.tile([128, HPC, D], BF16, tag="wo")
            masks_t = glob.tile([128, nm, QTILE], BF16, tag="masks")
            ones_t = glob.tile([128, 1], BF16, tag="ones")
            onesr_t = glob.tile([1, 128], F32R, tag="onesr")

            xT3 = xT.rearrange("(c p) s -> p c s", p=128)

            # ---------------- Phase 1: projections + rope ----------------
            # ps1 (k/v) created before ps1q (q) so phase-2 pools alias the
            # banks freed earliest at the tail (k ropes run during sb7's V
            # loop; q ropes are the last DVE ops).
            with tc.tile_pool(name="wts", bufs=1) as wts, \
                 tc.tile_pool(name="xts", bufs=8) as xts, \
                 tc.tile_pool(name="tmp", bufs=3) as tmp, \
                 tc.tile_pool(name="ps1", bufs=1, space="PSUM") as ps1, \
                 tc.tile_pool(name="ps1q", bufs=2, space="PSUM") as ps1q:
                wqkv_t = wts.tile([128, 3, KC, DCOL], BF16, tag="wqkv")

                def load_w(kind, g):
                    nc.sync.dma_start(
                        out=wqkv_t[:, kind, g * XG:(g + 1) * XG],
                        in_=wqkv[kind, g * XG:(g + 1) * XG].rearrange("c p w -> p c w"))

                xt_tiles = {}   # pair -> list of group tiles [128, XG, 1024]

                def load_group(pair, g):
                    t = xts.tile([128, XG, 2 * QTILE], BF16, tag="xt", name="xt")
                    nc.sync.dma_start(
                        out=t,
                        in_=xT3[:, g * XG:(g + 1) * XG,
                                pair * 2 * QTILE:(pair + 1) * 2 * QTILE])
                    xt_tiles.setdefault(pair, []).append(t)

                # cold-start critical DMA order: feed the Q loop first (wq
                # groups + xt groups), then wk, wv, then rope tables and
                # phase-2 constants.
                load_w(0, 0)
                load_group(0, 0)
                load_w(0, 1)
                load_w(0, 2)
                load_w(0, 3)
                load_group(0, 1)
                load_w(1, 0)
                load_w(1, 1)
                load_group(0, 2)
                load_w(1, 2)
                load_w(1, 3)
                load_group(0, 3)
                for g in range(4):
                    load_w(2, g)
                nc.sync.dma_start(out=cos2_t, in_=cos2[:, :])
                nc.sync.dma_start(out=sin2_t, in_=sin2[:, :])
                nc.sync.dma_start(out=wo_t, in_=wo.rearrange("j p d -> p j d"))
                nc.sync.dma_start(out=masks_t,
                                  in_=masks.rearrange("m p q -> p m q"))
                nc.sync.dma_start(out=ones_t, in_=ones[:, :])
                nc.sync.dma_start(out=onesr_t, in_=onesrow[:, :])

                for sb in range(NSB):
                    pair, half = sb // 2, sb % 2
                    if half == 0 and pair + 1 < NSB // 2:
                        for g in range(KC // XG):
                            load_group(pair + 1, g)
                    xg = xt_tiles[pair]

                    def xsl(c):
                        return xg[c // XG][:, c % XG, half * QTILE:(half + 1) * QTILE]

                    k_ps = [ps1.tile([128, QTILE], F32, tag=f"k{h}", name=f"k_ps{h}")
                            for h in range(HPC)]
                    v_ps = [ps1.tile([128, 2, DCOL], F32, tag=f"v{j}", name=f"v_ps{j}")
                            for j in range(2)]
                    q_ps = [ps1q.tile([128, QTILE], F32, tag=f"q{h}", name=f"q_ps{h}")
                            for h in range(HPC)]

                    scol = (sb * QTILE) % S
                    cs = cos2_t[:, scol:scol + QTILE]
                    sn = sin2_t[:, scol:scol + QTILE]

                    def rope(ps, dst):
                        t1 = tmp.tile([128, QTILE], F32, tag="t1", name="t1")
                        nc.vector.tensor_mul(t1, ps, cs)
                        nc.vector.tensor_mul(ps, ps, sn)
                        nc.vector.tensor_sub(dst[0:64, :], t1[0:64, :], ps[64:128, :])
                        nc.vector.tensor_add(dst[64:128, :], ps[0:64, :], t1[64:128, :])

                    # Q loop
                    for c in range(KC):
                        for h in range(HPC):
                            nc.tensor.matmul(q_ps[h],
                                             wqkv_t[:, 0, c, h * HD:(h + 1) * HD],
                                             xsl(c), start=(c == 0), stop=(c == KC - 1))
                    # K loop
                    for c in range(KC):
                        for h in range(HPC):
                            nc.tensor.matmul(k_ps[h],
                                             wqkv_t[:, 1, c, h * HD:(h + 1) * HD],
                                             xsl(c), start=(c == 0), stop=(c == KC - 1))
                    # K rope drains k_ps on DVE while the PE streams the V loop
                    for h in range(HPC):
                        rope(k_ps[h], KTs[sb][:, h, :])
                    # V loop (xt slices as stationary -> natural-layout V)
                    for c in range(KC):
                        x512 = xsl(c)
                        for sub in range(4):
                            nc.tensor.matmul(v_ps[sub // 2][:, sub % 2, :],
                                             x512[:, sub * 128:(sub + 1) * 128],
                                             wqkv_t[:, 2, c, :],
                                             start=(c == 0 and sub % 2 == 0),
                                             stop=(c == KC - 1),
                                             skip_group_check=True)
                    for j in range(2):
                        nc.vector.tensor_copy(
                            VSs[sb][:, 2 * j:2 * j + 2, :].rearrange("p a b -> p (a b)"),
                            v_ps[j].rearrange("p a b -> p (a b)"))
                    for h in range(HPC):
                        rope(q_ps[h], QTs[sb][:, h, :])

            # ---------------- Phase 2 + 3 interleaved ----------------
            with tc.tile_pool(name="pts", bufs=6) as pts, \
                 tc.tile_pool(name="rbs", bufs=2) as rbs, \
                 tc.tile_pool(name="outs", bufs=3) as outs, \
                 tc.tile_pool(name="ps_s", bufs=3, space="PSUM") as ps_s, \
                 tc.tile_pool(name="ps_o", bufs=2, space="PSUM") as ps_o, \
                 tc.tile_pool(name="ps_r", bufs=1, space="PSUM") as ps_r, \
                 tc.tile_pool(name="ps3", bufs=2, space="PSUM") as ps3:

                # Deferred-op scheduler (PE-queue-blocking pieces emitted a few
                # units after their inputs start so the in-order PE stream
                # never waits on DVE/ACT latency).
                import heapq
                todo = []
                gstep = 0
                seq = [0]

                def sched(delay, fn):
                    heapq.heappush(todo, (gstep + delay, seq[0], fn))
                    seq[0] += 1

                def emit_due():
                    while todo and todo[0][0] <= gstep:
                        heapq.heappop(todo)[2]()

                def wo_part(sbq, w):
                    os = outs.tile([128, 4, QTILE], BF16, tag="os", name="os")
                    for i in range(4):
                        dc = w * 4 + i
                        po3 = ps3.tile([128, QTILE], F32, tag="x", name="po3")
                        for j in range(HPC):
                            nc.tensor.matmul(po3, wo_t[:, j, dc * 128:(dc + 1) * 128],
                                             OTs[sbq][:, j, :],
                                             start=(j == 0), stop=(j == HPC - 1))
                        nc.any.tensor_copy(os[:, i, :], po3)
                    nc.sync.dma_start(
                        out=outT[w * 4:(w + 1) * 4, :, sbq * QTILE:(sbq + 1) * QTILE]
                            .rearrange("g p s -> p g s"),
                        in_=os)

                def norm_chain(h, sbq, po, pr, is_last_head):
                    # reciprocal first: frees pr for the next chain's rowsum
                    rpf = rbs.tile([1, QTILE], F32, tag="rpf", name="rpf")
                    nc.vector.reciprocal_approx_fast(rpf, pr)
                    rp = rbs.tile([1, QTILE], F32R, tag="rp", name="rp")
                    with nc.allow_low_precision(reason="fp32r bits are fp32"):
                        nc.vector.tensor_copy(rp, rpf)

                    def _b():
                        rb_ps = ps3.tile([128, QTILE], F32, tag="x", name="rb_ps")
                        nc.tensor.matmul(rb_ps, onesr_t[:, :], rp[:, :],
                                         start=True, stop=True)
                        rb = rbs.tile([128, QTILE], F32, tag="rb", name="rb")
                        nc.scalar.copy(rb, rb_ps)

                        def _c():
                            nc.vector.tensor_mul(OTs[sbq][:, h, :], po, rb)
                            if is_last_head:
                                for w in range(4):
                                    sched(1 + w, lambda w=w: wo_part(sbq, w))
                        sched(1, _c)
                    sched(1, _b)

                # flat unit stream over all (b,qt,h,block); ends on a short
                # chain so the un-overlapped tail is small
                bqt_order = [(0, 0), (0, 1), (0, 2), (0, 3),
                             (1, 1), (1, 2), (1, 3), (1, 0)]
                units = []
                for b, qt in bqt_order:
                    sbq = b * NQT + qt
                    blist = blocks[qt]
                    for h in range(HPC):
                        for i, (kt, mid) in enumerate(blist):
                            units.append((b, qt, sbq, h, i, kt, mid, len(blist)))

                state = {}      # (sbq,h) -> (po, pr)
                pt_of = {}      # unit idx -> pt tile

                def emit_score(idx):
                    b, qt, sbq, h, i, kt, mid, nbl = units[idx]
                    sbk = (b * S + kt * 128) // QTILE
                    ck = (kt * 128) % QTILE
                    s_ps = ps_s.tile([128, QTILE], F32, tag="s", name="s_ps")
                    nc.tensor.matmul(s_ps, KTs[sbk][:, h, ck:ck + 128],
                                     QTs[sbq][:, h, :], start=True, stop=True)
                    pt = pts.tile([128, QTILE], BF16, tag="pt", name="pt")
                    nc.scalar.activation(pt, s_ps,
                                         mybir.ActivationFunctionType.Exp,
                                         scale=float(ISQ))
                    if mid is not None:
                        nc.vector.tensor_mul(pt, pt, masks_t[:, mid, :])
                    pt_of[idx] = pt

                def pv_mm(idx):
                    b, qt, sbq, h, i, kt, mid, nbl = units[idx]
                    if i == 0:
                        po = ps_o.tile([128, QTILE], F32, tag="po", name="po")
                        pr = ps_r.tile([1, QTILE], F32, tag="pr", name="pr")
                        state[(sbq, h)] = (po, pr)
                    po, pr = state[(sbq, h)]
                    gkt = (b * S) // 128 + kt
                    nc.tensor.matmul(po, VSs[gkt // 4][:, gkt % 4, h * HD:(h + 1) * HD],
                                     pt_of[idx], start=(i == 0), stop=(i == nbl - 1))

                def rs_mm(idx):
                    b, qt, sbq, h, i, kt, mid, nbl = units[idx]
                    po, pr = state[(sbq, h)]
                    nc.tensor.matmul(pr, ones_t[:, :], pt_of.pop(idx),
                                     start=(i == 0), stop=(i == nbl - 1))
                    if i == nbl - 1:
                        norm_chain(h, sbq, po, pr, h == HPC - 1)

                # pairwise emission: back-to-back same-target matmuls
                # (chain lengths are even, so pairs never straddle a chain)
                nu = len(units)
                for pidx in range(0, nu + LA, 2):
                    if pidx < nu:
                        emit_score(pidx)
                        emit_score(pidx + 1)
                    j = pidx - LA
                    if j >= 0:
                        pv_mm(j)
                        pv_mm(j + 1)
                        rs_mm(j)
                        rs_mm(j + 1)
                        gstep += 1
                        emit_due()
                while todo:
                    gstep += 1
                    emit_due()
    nc.compile()
    return nc


# ---------------------------------------------------------------------------
# Host-side preparation
# ---------------------------------------------------------------------------

_CACHE = {}


def _classify_blocks(mask):
    """mask: additive [S, S] (q, k) -> (blocks, mask_tiles[nm, 128, QTILE])."""
    nqt, nkb = S // QTILE, S // KBLK
    mult = np.exp(np.minimum(mask, 0.0).astype(np.float64)).astype(np.float32)
    blocks = []
    tiles = []
    tile_index = {}
    for qt in range(nqt):
        row = []
        qs = slice(qt * QTILE, (qt + 1) * QTILE)
        for kt in range(nkb):
            ks = slice(kt * KBLK, (kt + 1) * KBLK)
            blk = mult[qs, ks].T  # [k, q]
            if not blk.any():
                continue
            if (blk == 1.0).all():
                row.append((kt, None))
                continue
            key = blk.tobytes()
            if key not in tile_index:
                tile_index[key] = len(tiles)
                tiles.append(np.ascontiguousarray(blk))
            row.append((kt, tile_index[key]))
        assert row, "fully-masked q-tile row: softmax undefined in this kernel"
        blocks.append(row)
    if not tiles:
        tiles.append(np.zeros((KBLK, QTILE), np.float32))
    return blocks, np.stack(tiles, axis=0)


def _perm_even_odd():
    p = np.empty(HD, np.int64)
    p[:64] = np.arange(0, HD, 2)
    p[64:] = np.arange(1, HD, 2)
    return p


def _bf16(a):
    return np.ascontiguousarray(np.asarray(a, np.float32).astype(ml_dtypes.bfloat16))


def kernel(x, wq, wk, wv, wo, wq_A, wq_B, wk_A, wk_B, wv_A, wv_B,
           wo_A, wo_B, cos, sin, mask):
    x = np.asarray(x, np.float32)
    to64 = lambda a: np.asarray(a, np.float32).astype(np.float64)

    wq_f = (to64(wq) + SCALING * (to64(wq_A) @ to64(wq_B))).astype(np.float32)
    wk_f = (to64(wk) + SCALING * (to64(wk_A) @ to64(wk_B))).astype(np.float32)
    wv_f = (to64(wv) + SCALING * (to64(wv_A) @ to64(wv_B))).astype(np.float32)
    wo_f = (to64(wo) + SCALING * (to64(wo_A) @ to64(wo_B))).astype(np.float32)

    perm = _perm_even_odd()
    full_perm = np.concatenate([h * HD + perm for h in range(H)])
    wq_p = wq_f[:, full_perm]
    wk_p = wk_f[:, full_perm]

    cosT = np.ascontiguousarray(np.asarray(cos, np.float32).T)  # [64, S]
    sinT = np.ascontiguousarray(np.asarray(sin, np.float32).T)
    cos2 = np.concatenate([cosT, cosT], axis=0)  # [128, S]
    sin2 = np.concatenate([sinT, sinT], axis=0)

    m2d = np.asarray(mask, np.float32).reshape(S, S)
    blocks, mask_tiles = _classify_blocks(m2d)
    nm = mask_tiles.shape[0]

    sig = (tuple(tuple(r) for r in blocks), nm)
    if sig not in _CACHE:
        _CACHE[sig] = build_kernel(blocks, nm)
    nc = _CACHE[sig]

    xT = _bf16(x.reshape(SEQ, D).T)
    ones = np.ones((128, 1), ml_dtypes.bfloat16)
    onesrow = np.ones((1, 128), np.float32)

    in_maps = []
    for c in range(N_CORES):
        cols = slice(c * DCOL, (c + 1) * DCOL)
        wqkv = np.stack([
            _bf16(wq_p[:, cols]).reshape(KC, 128, DCOL),
            _bf16(wk_p[:, cols]).reshape(KC, 128, DCOL),
            _bf16(wv_f[:, cols]).reshape(KC, 128, DCOL),
        ], axis=0)  # [3, KC, 128, DCOL]
        in_maps.append(dict(
            xT=xT,
            wqkv=wqkv,
            wo=_bf16(wo_f[cols, :]).reshape(HPC, 128, D),
            cos2=cos2, sin2=sin2, masks=_bf16(mask_tiles),
            ones=ones, onesrow=onesrow,
        ))

    global _LAST
    res = run_bass_kernel_spmd(nc, in_maps, list(range(N_CORES)), trace=_TRACE)
    _LAST = res
    acc = np.zeros((D, SEQ), np.float64)
    for r in res.results:
        acc += r["outT"].astype(np.float64).reshape(D, SEQ)
    return np.ascontiguousarray(acc.T.astype(np.float32)).reshape(B, S, D)


_TRACE = False   # test harness can set kernel._TRACE = True to profile
_LAST = None     # last BassKernelResults (exec_time_ns when traced)


# revision 19
# speedup vs baseline: 1.2653x; 1.0764x over previous
"""Trainium2 Bass kernel for LoRA multi-head causal attention (tensor-parallel
over heads across 8 NeuronCores).

Math (per reference):
  q = x@wq + (x@wq_A)@wq_B * 2 ; k,v analogous ; rope(q,k) ; causal softmax
  attention ; out = a@wo + (a@wo_A)@wo_B * 2

Device strategy (per core c, heads 2c and 2c+1):
  - LoRA folded into the dense weights on host (x@W + (x@A)@B*s == x@(W+s*A@B)).
  - q/k weights column-permuted per head (even rope pairs first) so rope is
    contiguous half-tile arithmetic; QK^T is invariant under a shared head-dim
    permutation.
  - Matmul operands bf16; accumulation/softmax fp32 in PSUM; normalizer path
    float32r.
  - Phase 1 (per 512-token sb tile, b-major): three separated chunk loops
    Q(c0..15), K(c0..15), V(c0..15) so each PSUM consumer (rope on DVE, V
    copies) drains while the PE streams the next loop.  QT/KT/VS are per-sb
    tiles so phase-2 dependencies are precise; PSUM pool creation order makes
    phase-2 banks alias the earliest-freed phase-1 banks.
  - DMA descriptor count minimized (sync-engine descriptor programming is
    ~600ns each and was the phase-1 supply bottleneck): weights one
    multi-chunk strided descriptor per kind, xt in 4-chunk x 1024-col groups,
    outputs in 4-block groups; issue order hand-arranged for the cold start.
  - Phase 2: flat stream over all (b,qt,h,block) units with lookahead-2
    software pipelining: scores for block i+2 are emitted before PV/rowsum of
    block i, so the PE never waits on the ScalarE exp.  Single-block exp
    (PSUM->SBUF bf16, scale=1/sqrt(hd)); multiplicative mask tiles on partial
    blocks; rowsum via ones-vector matmul; normalization reciprocal on DVE,
    broadcast via K=1 matmul; wo projection interleaved via deferred
    scheduler; partial outputs written bf16 (host does the fp32 all-reduce).
"""
import sys
import math

sys.path.insert(0, "/opt/trn_rl_repo")

import numpy as np
import ml_dtypes

import concourse.bass as bass
from concourse import bacc
import concourse.mybir as mybir
from concourse.tile import TileContext
from concourse.bass_utils import run_bass_kernel_spmd

F32 = mybir.dt.float32
F32R = mybir.dt.float32r
BF16 = mybir.dt.bfloat16

B, S, D, H, R = 2, 2048, 2048, 16, 8
HD = D // H                     # 128
SCALING = 16.0 / R              # 2.0
N_CORES = 8
HPC = H // N_CORES              # heads per core = 2
DCOL = HPC * HD                 # per-core projection width = 256
SEQ = B * S                     # 4096
ISQ = 1.0 / math.sqrt(HD)
QTILE = 512                     # q-tile width (free dim)
KBLK = 128                      # k-block (partition dim)
KC = D // 128                   # 16 contraction chunks
NSB = SEQ // QTILE              # 8 sb tiles
NQT = S // QTILE                # 4 q tiles per batch
LA = 2                          # phase-2 score lookahead (blocks)
XG = 4                          # xt chunks per DMA group


def build_kernel(blocks, nm):
    nc = bacc.Bacc("TRN2", target_bir_lowering=False, debug=False)

    xT = nc.declare_dram_parameter("xT", [D, SEQ], BF16, isOutput=False)
    wqkv = nc.declare_dram_parameter("wqkv", [3, KC, 128, DCOL], BF16, isOutput=False)
    wo = nc.declare_dram_parameter("wo", [HPC, 128, D], BF16, isOutput=False)
    cos2 = nc.declare_dram_parameter("cos2", [HD, S], F32, isOutput=False)
    sin2 = nc.declare_dram_parameter("sin2", [HD, S], F32, isOutput=False)
    masks = nc.declare_dram_parameter("masks", [nm, 128, QTILE], BF16, isOutput=False)
    ones = nc.declare_dram_parameter("ones", [128, 128], BF16, isOutput=False)
    outT = nc.declare_dram_parameter("outT", [KC, 128, SEQ], BF16, isOutput=True)

    with TileContext(nc) as tc:
        from contextlib import ExitStack
        with ExitStack() as top:
            glob = top.enter_context(tc.tile_pool(name="glob", bufs=1))
            qkvs = top.enter_context(tc.tile_pool(name="qkvs", bufs=1))

            QTs = [qkvs.tile([128, HPC, QTILE], BF16, tag=f"QT{sb}", name=f"QT{sb}")
                   for sb in range(NSB)]
            KTs = [qkvs.tile([128, HPC, QTILE], BF16, tag=f"KT{sb}", name=f"KT{sb}")
                   for sb in range(NSB)]
            VSs = [qkvs.tile([128, QTILE // 128, DCOL], BF16, tag=f"VS{sb}", name=f"VS{sb}")
                   for sb in range(NSB)]
            OTs = [qkvs.tile([128, HPC, QTILE], BF16, tag=f"OT{sb}", name=f"OT{sb}")
                   for sb in range(NSB)]

            cos2_t = glob.tile([HD, S], F32, tag="cos2")
            sin2_t = glob.tile([HD, S], F32, tag="sin2")
            wo_t = glob.tile([128, HPC, D], BF16, tag="wo")
            masks_t = glob.tile([128, nm, QTILE], BF16, tag="masks")
            ones_t = glob.tile([128, 128], BF16, tag="ones")

            xT3 = xT.rearrange("(c p) s -> p c s", p=128)

            # ---------------- Phase 1: projections + rope ----------------
            # ps1 (k/v) created before ps1q (q) so phase-2 pools alias the
            # banks freed earliest at the tail (k ropes run during sb7's V
            # loop; q ropes are the last DVE ops).
            with tc.tile_pool(name="wts", bufs=1) as wts, \
                 tc.tile_pool(name="xts", bufs=8) as xts, \
                 tc.tile_pool(name="tmp", bufs=3) as tmp, \
                 tc.tile_pool(name="ps1", bufs=1, space="PSUM") as ps1, \
                 tc.tile_pool(name="ps1q", bufs=2, space="PSUM") as ps1q:
                wqkv_t = wts.tile([128, 3, KC, DCOL], BF16, tag="wqkv")

                def load_w(kind, g):
                    nc.sync.dma_start(
                        out=wqkv_t[:, kind, g * XG:(g + 1) * XG],
                        in_=wqkv[kind, g * XG:(g + 1) * XG].rearrange("c p w -> p c w"))

                xt_tiles = {}   # pair -> list of group tiles [128, XG, 1024]

                def load_group(pair, g):
                    t = xts.tile([128, XG, 2 * QTILE], BF16, tag="xt", name="xt")
                    nc.sync.dma_start(
                        out=t,
                        in_=xT3[:, g * XG:(g + 1) * XG,
                                pair * 2 * QTILE:(pair + 1) * 2 * QTILE])
                    xt_tiles.setdefault(pair, []).append(t)

                # cold-start critical DMA order: feed the Q loop first (wq
                # groups + xt groups), then wk, wv, then rope tables and
                # phase-2 constants.
                load_w(0, 0)
                load_group(0, 0)
                load_w(0, 1)
                load_w(0, 2)
                load_w(0, 3)
                load_group(0, 1)
                load_w(1, 0)
                load_w(1, 1)
                load_group(0, 2)
                load_w(1, 2)
                load_w(1, 3)
                load_group(0, 3)
                for g in range(4):
                    load_w(2, g)
                nc.sync.dma_start(out=cos2_t, in_=cos2[:, :])
                nc.sync.dma_start(out=sin2_t, in_=sin2[:, :])
                nc.sync.dma_start(out=wo_t, in_=wo.rearrange("j p d -> p j d"))
                nc.sync.dma_start(out=masks_t,
                                  in_=masks.rearrange("m p q -> p m q"))
                nc.sync.dma_start(out=ones_t, in_=ones[:, :])

                for sb in range(NSB):
                    pair, half = sb // 2, sb % 2
                    if half == 0 and pair + 1 < NSB // 2:
                        for g in range(KC // XG):
                            load_group(pair + 1, g)
                    xg = xt_tiles[pair]

                    def xsl(c):
                        return xg[c // XG][:, c % XG, half * QTILE:(half + 1) * QTILE]

                    k_ps = [ps1.tile([128, QTILE], F32, tag=f"k{h}", name=f"k_ps{h}")
                            for h in range(HPC)]
                    v_ps = [ps1.tile([128, 2, DCOL], F32, tag=f"v{j}", name=f"v_ps{j}")
                            for j in range(2)]
                    q_ps = [ps1q.tile([128, QTILE], F32, tag=f"q{h}", name=f"q_ps{h}")
                            for h in range(HPC)]

                    scol = (sb * QTILE) % S
                    cs = cos2_t[:, scol:scol + QTILE]
                    sn = sin2_t[:, scol:scol + QTILE]

                    def rope(ps, dst):
                        t1 = tmp.tile([128, QTILE], F32, tag="t1", name="t1")
                        nc.vector.tensor_mul(t1, ps, cs)
                        nc.vector.tensor_mul(ps, ps, sn)
                        nc.vector.tensor_sub(dst[0:64, :], t1[0:64, :], ps[64:128, :])
                        nc.vector.tensor_add(dst[64:128, :], ps[0:64, :], t1[64:128, :])

                    # Q loop
                    for c in range(KC):
                        for h in range(HPC):
                            nc.tensor.matmul(q_ps[h],
                                             wqkv_t[:, 0, c, h * HD:(h + 1) * HD],
                                             xsl(c), start=(c == 0), stop=(c == KC - 1))
                    # K loop
                    for c in range(KC):
                        for h in range(HPC):
                            nc.tensor.matmul(k_ps[h],
                                             wqkv_t[:, 1, c, h * HD:(h + 1) * HD],
                                             xsl(c), start=(c == 0), stop=(c == KC - 1))
                    # K rope drains k_ps on DVE while the PE streams the V loop
                    for h in range(HPC):
                        rope(k_ps[h], KTs[sb][:, h, :])
                    # V loop (xt slices as stationary -> natural-layout V)
                    for c in range(KC):
                        x512 = xsl(c)
                        for sub in range(4):
                            nc.tensor.matmul(v_ps[sub // 2][:, sub % 2, :],
                                             x512[:, sub * 128:(sub + 1) * 128],
                                             wqkv_t[:, 2, c, :],
                                             start=(c == 0 and sub % 2 == 0),
                                             stop=(c == KC - 1),
                                             skip_group_check=True)
                    for j in range(2):
                        nc.vector.tensor_copy(
                            VSs[sb][:, 2 * j:2 * j + 2, :].rearrange("p a b -> p (a b)"),
                            v_ps[j].rearrange("p a b -> p (a b)"))
                    for h in range(HPC):
                        rope(q_ps[h], QTs[sb][:, h, :])

            # ---------------- Phase 2 + 3 interleaved ----------------
            with tc.tile_pool(name="pts", bufs=6) as pts, \
                 tc.tile_pool(name="rbs", bufs=2) as rbs, \
                 tc.tile_pool(name="outs", bufs=3) as outs, \
                 tc.tile_pool(name="ps_s", bufs=3, space="PSUM") as ps_s, \
                 tc.tile_pool(name="ps_o", bufs=2, space="PSUM") as ps_o, \
                 tc.tile_pool(name="ps_r", bufs=1, space="PSUM") as ps_r, \
                 tc.tile_pool(name="ps3", bufs=2, space="PSUM") as ps3:

                # Deferred-op scheduler (PE-queue-blocking pieces emitted a few
                # units after their inputs start so the in-order PE stream
                # never waits on DVE/ACT latency).
                import heapq
                todo = []
                gstep = 0
                seq = [0]

                def sched(delay, fn):
                    heapq.heappush(todo, (gstep + delay, seq[0], fn))
                    seq[0] += 1

                def emit_due():
                    while todo and todo[0][0] <= gstep:
                        heapq.heappop(todo)[2]()

                def wo_part(sbq, w):
                    os = outs.tile([128, 4, QTILE], BF16, tag="os", name="os")
                    for i in range(4):
                        dc = w * 4 + i
                        po3 = ps3.tile([128, QTILE], F32, tag="x", name="po3")
                        for j in range(HPC):
                            nc.tensor.matmul(po3, wo_t[:, j, dc * 128:(dc + 1) * 128],
                                             OTs[sbq][:, j, :],
                                             start=(j == 0), stop=(j == HPC - 1))
                        nc.any.tensor_copy(os[:, i, :], po3)
                    nc.sync.dma_start(
                        out=outT[w * 4:(w + 1) * 4, :, sbq * QTILE:(sbq + 1) * QTILE]
                            .rearrange("g p s -> p g s"),
                        in_=os)

                def norm_chain(h, sbq, po, pr, is_last_head):
                    # pr is the denominator already broadcast across all 128
                    # partitions (ones lhsT is [128,128]); reciprocal first
                    # frees pr for the next chain's rowsum
                    rbf = rbs.tile([128, QTILE], F32, tag="rbf", name="rbf")
                    nc.vector.reciprocal_approx_fast(rbf, pr)

                    def _c():
                        nc.vector.tensor_mul(OTs[sbq][:, h, :], po, rbf)
                        if is_last_head:
                            for w in range(4):
                                sched(1 + w, lambda w=w: wo_part(sbq, w))
                    sched(1, _c)

                # flat unit stream over all (b,qt,h,block); starts with a
                # chain whose first blocks are unmasked (no DVE dependency)
                # and ends on a short chain so the un-overlapped tail is small
                bqt_order = [(0, 1), (0, 0), (0, 2), (0, 3),
                             (1, 1), (1, 2), (1, 3), (1, 0)]
                units = []
                for b, qt in bqt_order:
                    sbq = b * NQT + qt
                    blist = blocks[qt]
                    for h in range(HPC):
                        for i, (kt, mid) in enumerate(blist):
                            units.append((b, qt, sbq, h, i, kt, mid, len(blist)))

                state = {}      # (sbq,h) -> (po, pr)
                pt_of = {}      # unit idx -> pt tile

                def emit_score(idx):
                    b, qt, sbq, h, i, kt, mid, nbl = units[idx]
                    sbk = (b * S + kt * 128) // QTILE
                    ck = (kt * 128) % QTILE
                    s_ps = ps_s.tile([128, QTILE], F32, tag="s", name="s_ps")
                    nc.tensor.matmul(s_ps, KTs[sbk][:, h, ck:ck + 128],
                                     QTs[sbq][:, h, :], start=True, stop=True)
                    pt = pts.tile([128, QTILE], BF16, tag="pt", name="pt")
                    nc.scalar.activation(pt, s_ps,
                                         mybir.ActivationFunctionType.Exp,
                                         scale=float(ISQ))
                    if mid is not None:
                        nc.vector.tensor_mul(pt, pt, masks_t[:, mid, :])
                    pt_of[idx] = pt

                def pv_mm(idx):
                    b, qt, sbq, h, i, kt, mid, nbl = units[idx]
                    if i == 0:
                        po = ps_o.tile([128, QTILE], F32, tag="po", name="po")
                        pr = ps_r.tile([128, QTILE], F32, tag="pr", name="pr")
                        state[(sbq, h)] = (po, pr)
                    po, pr = state[(sbq, h)]
                    gkt = (b * S) // 128 + kt
                    nc.tensor.matmul(po, VSs[gkt // 4][:, gkt % 4, h * HD:(h + 1) * HD],
                                     pt_of[idx], start=(i == 0), stop=(i == nbl - 1))

                def rs_mm(idx):
                    b, qt, sbq, h, i, kt, mid, nbl = units[idx]
                    po, pr = state[(sbq, h)]
                    nc.tensor.matmul(pr, ones_t[:, :], pt_of.pop(idx),
                                     start=(i == 0), stop=(i == nbl - 1))
                    if i == nbl - 1:
                        norm_chain(h, sbq, po, pr, h == HPC - 1)

                # pairwise emission: back-to-back same-target matmuls
                # (chain lengths are even, so pairs never straddle a chain)
                nu = len(units)
                for pidx in range(0, nu + LA, 2):
                    if pidx < nu:
                        emit_score(pidx)
                        emit_score(pidx + 1)
                    j = pidx - LA
                    if j >= 0:
                        pv_mm(j)
                        pv_mm(j + 1)
                        rs_mm(j)
                        rs_mm(j + 1)
                        gstep += 1
                        emit_due()
                while todo:
                    gstep += 1
                    emit_due()
    nc.compile()
    return nc


# ---------------------------------------------------------------------------
# Host-side preparation
# ---------------------------------------------------------------------------

_CACHE = {}


def _classify_blocks(mask):
    """mask: additive [S, S] (q, k) -> (blocks, mask_tiles[nm, 128, QTILE])."""
    nqt, nkb = S // QTILE, S // KBLK
    mult = np.exp(np.minimum(mask, 0.0).astype(np.float64)).astype(np.float32)
    blocks = []
    tiles = []
    tile_index = {}
    for qt in range(nqt):
        row = []
        qs = slice(qt * QTILE, (qt + 1) * QTILE)
        for kt in range(nkb):
            ks = slice(kt * KBLK, (kt + 1) * KBLK)
            blk = mult[qs, ks].T  # [k, q]
            if not blk.any():
                continue
            if (blk == 1.0).all():
                row.append((kt, None))
                continue
            key = blk.tobytes()
            if key not in tile_index:
                tile_index[key] = len(tiles)
                tiles.append(np.ascontiguousarray(blk))
            row.append((kt, tile_index[key]))
        assert row, "fully-masked q-tile row: softmax undefined in this kernel"
        blocks.append(row)
    if not tiles:
        tiles.append(np.zeros((KBLK, QTILE), np.float32))
    return blocks, np.stack(tiles, axis=0)


def _perm_even_odd():
    p = np.empty(HD, np.int64)
    p[:64] = np.arange(0, HD, 2)
    p[64:] = np.arange(1, HD, 2)
    return p


def _bf16(a):
    return np.ascontiguousarray(np.asarray(a, np.float32).astype(ml_dtypes.bfloat16))


def kernel(x, wq, wk, wv, wo, wq_A, wq_B, wk_A, wk_B, wv_A, wv_B,
           wo_A, wo_B, cos, sin, mask):
    x = np.asarray(x, np.float32)
    to64 = lambda a: np.asarray(a, np.float32).astype(np.float64)

    wq_f = (to64(wq) + SCALING * (to64(wq_A) @ to64(wq_B))).astype(np.float32)
    wk_f = (to64(wk) + SCALING * (to64(wk_A) @ to64(wk_B))).astype(np.float32)
    wv_f = (to64(wv) + SCALING * (to64(wv_A) @ to64(wv_B))).astype(np.float32)
    wo_f = (to64(wo) + SCALING * (to64(wo_A) @ to64(wo_B))).astype(np.float32)

    perm = _perm_even_odd()
    full_perm = np.concatenate([h * HD + perm for h in range(H)])
    wq_p = wq_f[:, full_perm]
    wk_p = wk_f[:, full_perm]

    cosT = np.ascontiguousarray(np.asarray(cos, np.float32).T)  # [64, S]
    sinT = np.ascontiguousarray(np.asarray(sin, np.float32).T)
    cos2 = np.concatenate([cosT, cosT], axis=0)  # [128, S]
    sin2 = np.concatenate([sinT, sinT], axis=0)

    m2d = np.asarray(mask, np.float32).reshape(S, S)
    blocks, mask_tiles = _classify_blocks(m2d)
    nm = mask_tiles.shape[0]

    sig = (tuple(tuple(r) for r in blocks), nm)
    if sig not in _CACHE:
        _CACHE[sig] = build_kernel(blocks, nm)
    nc = _CACHE[sig]

    xT = _bf16(x.reshape(SEQ, D).T)
    ones = np.ones((128, 128), ml_dtypes.bfloat16)

    in_maps = []
    for c in range(N_CORES):
        cols = slice(c * DCOL, (c + 1) * DCOL)
        wqkv = np.stack([
            _bf16(wq_p[:, cols]).reshape(KC, 128, DCOL),
            _bf16(wk_p[:, cols]).reshape(KC, 128, DCOL),
            _bf16(wv_f[:, cols]).reshape(KC, 128, DCOL),
        ], axis=0)  # [3, KC, 128, DCOL]
        in_maps.append(dict(
            xT=xT,
            wqkv=wqkv,
            wo=_bf16(wo_f[cols, :]).reshape(HPC, 128, D),
            cos2=cos2, sin2=sin2, masks=_bf16(mask_tiles),
            ones=ones,
        ))

    global _LAST
    res = run_bass_kernel_spmd(nc, in_maps, list(range(N_CORES)), trace=_TRACE)
    _LAST = res
    acc = np.zeros((D, SEQ), np.float64)
    for r in res.results:
        acc += r["outT"].astype(np.float64).reshape(D, SEQ)
    return np.ascontiguousarray(acc.T.astype(np.float32)).reshape(B, S, D)


_TRACE = False   # test harness can set kernel._TRACE = True to profile
_LAST = None     # last BassKernelResults (exec_time_ns when traced)


# revision 25
# speedup vs baseline: 1.2754x; 1.0080x over previous
"""Trainium2 Bass kernel for LoRA multi-head causal attention (tensor-parallel
over heads across 8 NeuronCores).

Math (per reference):
  q = x@wq + (x@wq_A)@wq_B * 2 ; k,v analogous ; rope(q,k) ; causal softmax
  attention ; out = a@wo + (a@wo_A)@wo_B * 2

Device strategy (per core c, heads 2c and 2c+1):
  - LoRA folded into the dense weights on host (x@W + (x@A)@B*s == x@(W+s*A@B)).
  - q/k weights column-permuted per head (even rope pairs first) so rope is
    contiguous half-tile arithmetic; QK^T is invariant under a shared head-dim
    permutation.
  - Matmul operands bf16; accumulation/softmax fp32 in PSUM; normalizer path
    float32r.
  - Phase 1 (per 512-token sb tile, b-major): three separated chunk loops
    Q(c0..15), K(c0..15), V(c0..15) so each PSUM consumer (rope on DVE, V
    copies) drains while the PE streams the next loop.  QT/KT/VS are per-sb
    tiles so phase-2 dependencies are precise; PSUM pool creation order makes
    phase-2 banks alias the earliest-freed phase-1 banks.
  - DMA descriptor count minimized (sync-engine descriptor programming is
    ~600ns each and was the phase-1 supply bottleneck): weights one
    multi-chunk strided descriptor per kind, xt in 4-chunk x 1024-col groups,
    outputs in 4-block groups; issue order hand-arranged for the cold start.
  - Phase 2: flat stream over all (b,qt,h,block) units with lookahead-2
    software pipelining: scores for block i+2 are emitted before PV/rowsum of
    block i, so the PE never waits on the ScalarE exp.  Single-block exp
    (PSUM->SBUF bf16, scale=1/sqrt(hd)); multiplicative mask tiles on partial
    blocks; rowsum via ones-vector matmul; normalization reciprocal on DVE,
    broadcast via K=1 matmul; wo projection interleaved via deferred
    scheduler; partial outputs written bf16 (host does the fp32 all-reduce).
"""
import sys
import math

sys.path.insert(0, "/opt/trn_rl_repo")

import numpy as np
import ml_dtypes

import concourse.bass as bass
from concourse import bacc
import concourse.mybir as mybir
from concourse.tile import TileContext
from concourse.bass_utils import run_bass_kernel_spmd

F32 = mybir.dt.float32
F32R = mybir.dt.float32r
BF16 = mybir.dt.bfloat16

B, S, D, H, R = 2, 2048, 2048, 16, 8
HD = D // H                     # 128
SCALING = 16.0 / R              # 2.0
N_CORES = 8
HPC = H // N_CORES              # heads per core = 2
DCOL = HPC * HD                 # per-core projection width = 256
SEQ = B * S                     # 4096
ISQ = 1.0 / math.sqrt(HD)
QTILE = 512                     # q-tile width (free dim)
KBLK = 128                      # k-block (partition dim)
KC = D // 128                   # 16 contraction chunks
NSB = SEQ // QTILE              # 8 sb tiles
NQT = S // QTILE                # 4 q tiles per batch
LA = 2                          # phase-2 score lookahead (blocks)
XG = 4                          # xt chunks per DMA group


def build_kernel(blocks, nm):
    nc = bacc.Bacc("TRN2", target_bir_lowering=False, debug=False)

    xT = nc.declare_dram_parameter("xT", [D, SEQ], BF16, isOutput=False)
    wqkv = nc.declare_dram_parameter("wqkv", [3, KC, 128, DCOL], BF16, isOutput=False)
    wo = nc.declare_dram_parameter("wo", [HPC, 128, D], BF16, isOutput=False)
    cos2 = nc.declare_dram_parameter("cos2", [HD, S], F32, isOutput=False)
    sin2 = nc.declare_dram_parameter("sin2", [HD, S], F32, isOutput=False)
    masks = nc.declare_dram_parameter("masks", [nm, 128, QTILE], BF16, isOutput=False)
    ones = nc.declare_dram_parameter("ones", [128, 128], BF16, isOutput=False)
    outT = nc.declare_dram_parameter("outT", [KC, 128, SEQ], BF16, isOutput=True)

    with TileContext(nc) as tc:
        from contextlib import ExitStack
        with ExitStack() as top:
            glob = top.enter_context(tc.tile_pool(name="glob", bufs=1))
            qkvs = top.enter_context(tc.tile_pool(name="qkvs", bufs=1))

            QTs = [qkvs.tile([128, HPC, QTILE], BF16, tag=f"QT{sb}", name=f"QT{sb}")
                   for sb in range(NSB)]
            KTs = [qkvs.tile([128, HPC, QTILE], BF16, tag=f"KT{sb}", name=f"KT{sb}")
                   for sb in range(NSB)]
            VSs = [qkvs.tile([128, QTILE // 128, DCOL], BF16, tag=f"VS{sb}", name=f"VS{sb}")
                   for sb in range(NSB)]
            OTs = [qkvs.tile([128, HPC, QTILE], BF16, tag=f"OT{sb}", name=f"OT{sb}")
                   for sb in range(NSB)]

            cos2_t = glob.tile([HD, S], F32, tag="cos2")
            sin2_t = glob.tile([HD, S], F32, tag="sin2")
            wo_t = glob.tile([128, HPC, D], BF16, tag="wo")
            masks_t = glob.tile([128, nm, QTILE], BF16, tag="masks")
            ones_t = glob.tile([128, 128], BF16, tag="ones")

            xT3 = xT.rearrange("(c p) s -> p c s", p=128)

            # ---------------- Phase 1: projections + rope ----------------
            # ps1 (k/v) created before ps1q (q) so phase-2 pools alias the
            # banks freed earliest at the tail (k ropes run during sb7's V
            # loop; q ropes are the last DVE ops).
            with tc.tile_pool(name="wts", bufs=1) as wts, \
                 tc.tile_pool(name="xts", bufs=8) as xts, \
                 tc.tile_pool(name="tmp", bufs=3) as tmp, \
                 tc.tile_pool(name="ps1", bufs=1, space="PSUM") as ps1, \
                 tc.tile_pool(name="ps1q", bufs=2, space="PSUM") as ps1q:
                wqkv_t = wts.tile([128, 3, KC, DCOL], BF16, tag="wqkv")

                def load_w(kind, g):
                    nc.sync.dma_start(
                        out=wqkv_t[:, kind, g * XG:(g + 1) * XG],
                        in_=wqkv[kind, g * XG:(g + 1) * XG].rearrange("c p w -> p c w"))

                xt_tiles = {}   # pair -> list of group tiles [128, XG, 1024]

                def load_group(pair, g):
                    t = xts.tile([128, XG, 2 * QTILE], BF16, tag="xt", name="xt")
                    nc.sync.dma_start(
                        out=t,
                        in_=xT3[:, g * XG:(g + 1) * XG,
                                pair * 2 * QTILE:(pair + 1) * 2 * QTILE])
                    xt_tiles.setdefault(pair, []).append(t)

                # cold-start critical DMA order: feed the Q loop first (wq
                # groups + xt groups, fine-grained so the first matmuls start
                # as early as possible), then wk, wv, rope tables and phase-2
                # constants.
                nc.sync.dma_start(out=wqkv_t[:, 0, 0:2],
                                  in_=wqkv[0, 0:2].rearrange("c p w -> p c w"))
                t0 = xts.tile([128, XG, 2 * QTILE], BF16, tag="xt", name="xt")
                nc.sync.dma_start(out=t0[:, 0:2, :], in_=xT3[:, 0:2, 0:2 * QTILE])
                nc.sync.dma_start(out=wqkv_t[:, 0, 2:4],
                                  in_=wqkv[0, 2:4].rearrange("c p w -> p c w"))
                nc.sync.dma_start(out=t0[:, 2:4, :], in_=xT3[:, 2:4, 0:2 * QTILE])
                xt_tiles[0] = [t0]
                load_w(0, 1)
                load_w(0, 2)
                load_w(0, 3)
                load_group(0, 1)
                load_w(1, 0)
                load_w(1, 1)
                load_group(0, 2)
                load_w(1, 2)
                load_w(1, 3)
                load_group(0, 3)
                for g in range(4):
                    load_w(2, g)
                nc.sync.dma_start(out=cos2_t, in_=cos2[:, :])
                nc.sync.dma_start(out=sin2_t, in_=sin2[:, :])
                nc.sync.dma_start(out=wo_t, in_=wo.rearrange("j p d -> p j d"))
                nc.sync.dma_start(out=masks_t,
                                  in_=masks.rearrange("m p q -> p m q"))
                nc.sync.dma_start(out=ones_t, in_=ones[:, :])

                for sb in range(NSB):
                    pair, half = sb // 2, sb % 2
                    if half == 0 and pair + 1 < NSB // 2:
                        for g in range(KC // XG):
                            load_group(pair + 1, g)
                    xg = xt_tiles[pair]

                    def xsl(c):
                        return xg[c // XG][:, c % XG, half * QTILE:(half + 1) * QTILE]

                    k_ps = [ps1.tile([128, QTILE], F32, tag=f"k{h}", name=f"k_ps{h}")
                            for h in range(HPC)]
                    v_ps = [ps1.tile([128, 2, DCOL], F32, tag=f"v{j}", name=f"v_ps{j}")
                            for j in range(2)]
                    q_ps = [ps1q.tile([128, QTILE], F32, tag=f"q{h}", name=f"q_ps{h}")
                            for h in range(HPC)]

                    scol = (sb * QTILE) % S
                    cs = cos2_t[:, scol:scol + QTILE]
                    sn = sin2_t[:, scol:scol + QTILE]

                    def rope(ps, dst):
                        t1 = tmp.tile([128, QTILE], F32, tag="t1", name="t1")
                        nc.vector.tensor_mul(t1, ps, cs)
                        nc.vector.tensor_mul(ps, ps, sn)
                        nc.vector.tensor_sub(dst[0:64, :], t1[0:64, :], ps[64:128, :])
                        nc.vector.tensor_add(dst[64:128, :], ps[0:64, :], t1[64:128, :])

                    # Q loop
                    for c in range(KC):
                        for h in range(HPC):
                            nc.tensor.matmul(q_ps[h],
                                             wqkv_t[:, 0, c, h * HD:(h + 1) * HD],
                                             xsl(c), start=(c == 0), stop=(c == KC - 1))
                    if sb == NSB - 1:
                        # last sb: q-ropes early so the phase-2 pool boundary
                        # (which waits on the last phase-1 DVE op) isn't gated
                        # on a post-PE rope tail
                        for h in range(HPC):
                            rope(q_ps[h], QTs[sb][:, h, :])
                    # K loop
                    for c in range(KC):
                        for h in range(HPC):
                            nc.tensor.matmul(k_ps[h],
                                             wqkv_t[:, 1, c, h * HD:(h + 1) * HD],
                                             xsl(c), start=(c == 0), stop=(c == KC - 1))
                    # K rope drains k_ps on DVE while the PE streams the V loop
                    for h in range(HPC):
                        rope(k_ps[h], KTs[sb][:, h, :])
                    # V loop (xt slices as stationary -> natural-layout V)
                    for c in range(KC):
                        x512 = xsl(c)
                        for sub in range(4):
                            nc.tensor.matmul(v_ps[sub // 2][:, sub % 2, :],
                                             x512[:, sub * 128:(sub + 1) * 128],
                                             wqkv_t[:, 2, c, :],
                                             start=(c == 0 and sub % 2 == 0),
                                             stop=(c == KC - 1),
                                             skip_group_check=True)
                    for j in range(2):
                        nc.vector.tensor_copy(
                            VSs[sb][:, 2 * j:2 * j + 2, :].rearrange("p a b -> p (a b)"),
                            v_ps[j].rearrange("p a b -> p (a b)"))
                    if sb != NSB - 1:
                        for h in range(HPC):
                            rope(q_ps[h], QTs[sb][:, h, :])

            # ---------------- Phase 2 + 3 interleaved ----------------
            with tc.tile_pool(name="pts", bufs=6) as pts, \
                 tc.tile_pool(name="rbs", bufs=2) as rbs, \
                 tc.tile_pool(name="outs", bufs=3) as outs, \
                 tc.tile_pool(name="ps_s", bufs=3, space="PSUM") as ps_s, \
                 tc.tile_pool(name="ps_o", bufs=2, space="PSUM") as ps_o, \
                 tc.tile_pool(name="ps_r", bufs=1, space="PSUM") as ps_r, \
                 tc.tile_pool(name="ps3", bufs=2, space="PSUM") as ps3:

                # Deferred-op scheduler (PE-queue-blocking pieces emitted a few
                # units after their inputs start so the in-order PE stream
                # never waits on DVE/ACT latency).
                import heapq
                todo = []
                gstep = 0
                seq = [0]

                def sched(delay, fn):
                    heapq.heappush(todo, (gstep + delay, seq[0], fn))
                    seq[0] += 1

                def emit_due():
                    while todo and todo[0][0] <= gstep:
                        heapq.heappop(todo)[2]()

                def wo_part(sbq, w, split_dma=False):
                    os = outs.tile([128, 4, QTILE], BF16, tag="os", name="os")
                    for i in range(4):
                        dc = w * 4 + i
                        po3 = ps3.tile([128, QTILE], F32, tag="x", name="po3")
                        for j in range(HPC):
                            nc.tensor.matmul(po3, wo_t[:, j, dc * 128:(dc + 1) * 128],
                                             OTs[sbq][:, j, :],
                                             start=(j == 0), stop=(j == HPC - 1))
                        nc.any.tensor_copy(os[:, i, :], po3)
                        if split_dma:
                            nc.sync.dma_start(
                                out=outT[dc, :, sbq * QTILE:(sbq + 1) * QTILE],
                                in_=os[:, i, :])
                    if not split_dma:
                        nc.sync.dma_start(
                            out=outT[w * 4:(w + 1) * 4, :, sbq * QTILE:(sbq + 1) * QTILE]
                                .rearrange("g p s -> p g s"),
                            in_=os)

                def norm_chain(h, sbq, po, pr, is_last_head):
                    # pr is the denominator already broadcast across all 128
                    # partitions (ones lhsT is [128,128]); reciprocal first
                    # frees pr for the next chain's rowsum
                    rbf = rbs.tile([128, QTILE], F32, tag="rbf", name="rbf")
                    nc.vector.reciprocal_approx_fast(rbf, pr)

                    def _c():
                        nc.vector.tensor_mul(OTs[sbq][:, h, :], po, rbf)
                        if is_last_head:
                            last = sbq == LAST_SBQ
                            for w in range(4):
                                sched(1 + w, lambda w=w: wo_part(sbq, w, last and w == 3))
                    sched(1, _c)

                # flat unit stream over all (b,qt,h,block); starts with a
                # chain whose first blocks are unmasked (no DVE dependency)
                # and ends on a short chain so the un-overlapped tail is small
                bqt_order = [(0, 1), (0, 0), (0, 2), (0, 3),
                             (1, 1), (1, 2), (1, 3), (1, 0)]
                LAST_SBQ = bqt_order[-1][0] * NQT + bqt_order[-1][1]
                units = []
                for b, qt in bqt_order:
                    sbq = b * NQT + qt
                    blist = blocks[qt]
                    for h in range(HPC):
                        for i, (kt, mid) in enumerate(blist):
                            units.append((b, qt, sbq, h, i, kt, mid, len(blist)))

                state = {}      # (sbq,h) -> (po, pr)
                pt_of = {}      # unit idx -> pt tile

                def emit_score(idx):
                    b, qt, sbq, h, i, kt, mid, nbl = units[idx]
                    sbk = (b * S + kt * 128) // QTILE
                    ck = (kt * 128) % QTILE
                    s_ps = ps_s.tile([128, QTILE], F32, tag="s", name="s_ps")
                    nc.tensor.matmul(s_ps, KTs[sbk][:, h, ck:ck + 128],
                                     QTs[sbq][:, h, :], start=True, stop=True)
                    pt = pts.tile([128, QTILE], BF16, tag="pt", name="pt")
                    nc.scalar.activation(pt, s_ps,
                                         mybir.ActivationFunctionType.Exp,
                                         scale=float(ISQ))
                    if mid is not None:
                        nc.vector.tensor_mul(pt, pt, masks_t[:, mid, :])
                    pt_of[idx] = pt

                def pv_mm(idx):
                    b, qt, sbq, h, i, kt, mid, nbl = units[idx]
                    if i == 0:
                        po = ps_o.tile([128, QTILE], F32, tag="po", name="po")
                        pr = ps_r.tile([128, QTILE], F32, tag="pr", name="pr")
                        state[(sbq, h)] = (po, pr)
                    po, pr = state[(sbq, h)]
                    gkt = (b * S) // 128 + kt
                    nc.tensor.matmul(po, VSs[gkt // 4][:, gkt % 4, h * HD:(h + 1) * HD],
                                     pt_of[idx], start=(i == 0), stop=(i == nbl - 1))

                def rs_mm(idx):
                    b, qt, sbq, h, i, kt, mid, nbl = units[idx]
                    po, pr = state[(sbq, h)]
                    nc.tensor.matmul(pr, ones_t[:, :], pt_of.pop(idx),
                                     start=(i == 0), stop=(i == nbl - 1))
                    if i == nbl - 1:
                        norm_chain(h, sbq, po, pr, h == HPC - 1)

                # pairwise emission: back-to-back same-target matmuls
                # (chain lengths are even, so pairs never straddle a chain)
                nu = len(units)
                for pidx in range(0, nu + LA, 2):
                    if pidx < nu:
                        emit_score(pidx)
                        emit_score(pidx + 1)
                    j = pidx - LA
                    if j >= 0:
                        pv_mm(j)
                        pv_mm(j + 1)
                        rs_mm(j)
                        rs_mm(j + 1)
                        gstep += 1
                        emit_due()
                while todo:
                    gstep += 1
                    emit_due()
    nc.compile()
    return nc


# ---------------------------------------------------------------------------
# Host-side preparation
# ---------------------------------------------------------------------------

_CACHE = {}


def _classify_blocks(mask):
    """mask: additive [S, S] (q, k) -> (blocks, mask_tiles[nm, 128, QTILE])."""
    nqt, nkb = S // QTILE, S // KBLK
    mult = np.exp(np.minimum(mask, 0.0).astype(np.float64)).astype(np.float32)
    blocks = []
    tiles = []
    tile_index = {}
    for qt in range(nqt):
        row = []
        qs = slice(qt * QTILE, (qt + 1) * QTILE)
        for kt in range(nkb):
            ks = slice(kt * KBLK, (kt + 1) * KBLK)
            blk = mult[qs, ks].T  # [k, q]
            if not blk.any():
                continue
            if (blk == 1.0).all():
                row.append((kt, None))
                continue
            key = blk.tobytes()
            if key not in tile_index:
                tile_index[key] = len(tiles)
                tiles.append(np.ascontiguousarray(blk))
            row.append((kt, tile_index[key]))
        assert row, "fully-masked q-tile row: softmax undefined in this kernel"
        blocks.append(row)
    if not tiles:
        tiles.append(np.zeros((KBLK, QTILE), np.float32))
    return blocks, np.stack(tiles, axis=0)


def _perm_even_odd():
    p = np.empty(HD, np.int64)
    p[:64] = np.arange(0, HD, 2)
    p[64:] = np.arange(1, HD, 2)
    return p


def _bf16(a):
    return np.ascontiguousarray(np.asarray(a, np.float32).astype(ml_dtypes.bfloat16))


def kernel(x, wq, wk, wv, wo, wq_A, wq_B, wk_A, wk_B, wv_A, wv_B,
           wo_A, wo_B, cos, sin, mask):
    x = np.asarray(x, np.float32)
    to64 = lambda a: np.asarray(a, np.float32).astype(np.float64)

    wq_f = (to64(wq) + SCALING * (to64(wq_A) @ to64(wq_B))).astype(np.float32)
    wk_f = (to64(wk) + SCALING * (to64(wk_A) @ to64(wk_B))).astype(np.float32)
    wv_f = (to64(wv) + SCALING * (to64(wv_A) @ to64(wv_B))).astype(np.float32)
    wo_f = (to64(wo) + SCALING * (to64(wo_A) @ to64(wo_B))).astype(np.float32)

    perm = _perm_even_odd()
    full_perm = np.concatenate([h * HD + perm for h in range(H)])
    wq_p = wq_f[:, full_perm]
    wk_p = wk_f[:, full_perm]

    cosT = np.ascontiguousarray(np.asarray(cos, np.float32).T)  # [64, S]
    sinT = np.ascontiguousarray(np.asarray(sin, np.float32).T)
    cos2 = np.concatenate([cosT, cosT], axis=0)  # [128, S]
    sin2 = np.concatenate([sinT, sinT], axis=0)

    m2d = np.asarray(mask, np.float32).reshape(S, S)
    blocks, mask_tiles = _classify_blocks(m2d)
    nm = mask_tiles.shape[0]

    sig = (tuple(tuple(r) for r in blocks), nm)
    if sig not in _CACHE:
        _CACHE[sig] = build_kernel(blocks, nm)
    nc = _CACHE[sig]

    xT = _bf16(x.reshape(SEQ, D).T)
    ones = np.ones((128, 128), ml_dtypes.bfloat16)

    in_maps = []
    for c in range(N_CORES):
        cols = slice(c * DCOL, (c + 1) * DCOL)
        wqkv = np.stack([
            _bf16(wq_p[:, cols]).reshape(KC, 128, DCOL),
            _bf16(wk_p[:, cols]).reshape(KC, 128, DCOL),
            _bf16(wv_f[:, cols]).reshape(KC, 128, DCOL),
        ], axis=0)  # [3, KC, 128, DCOL]
        in_maps.append(dict(
            xT=xT,
            wqkv=wqkv,
            wo=_bf16(wo_f[cols, :]).reshape(HPC, 128, D),
            cos2=cos2, sin2=sin2, masks=_bf16(mask_tiles),
            ones=ones,
        ))

    global _LAST
    res = run_bass_kernel_spmd(nc, in_maps, list(range(N_CORES)), trace=_TRACE)
    _LAST = res
    acc = np.zeros((D, SEQ), np.float64)
    for r in res.results:
        acc += r["outT"].astype(np.float64).reshape(D, SEQ)
    return np.ascontiguousarray(acc.T.astype(np.float32)).reshape(B, S, D)


_TRACE = False   # test harness can set kernel._TRACE = True to profile
_LAST = None     # last BassKernelResults (exec_time_ns when traced)
